# revision 19
# baseline (speedup 1.0000x reference)
"""Trainium2 Bass kernel for nn_ColorHistogramMatchingLoss (v6).

Data-parallel over batch: core i processes image pair (x[i], y[i]) and emits
the per-image Hellinger distance; the host averages 8 scalars.

v6 reformulation (validated in numpy):
  - The three histograms reduce (via flip/transpose invariance of the loss)
    to G_uv = (iy Ru)^T Rv, G_uw = (iy Ru)^T Rw, G_wv = (iy Rw)^T Rv over the
    log-ratio fields u=lr-lg, w=lg-lb, v=lr-lb, and the weighted RBF column
    is iy * plain Lorentzian of the same field.
  - The PE matmul produces X = s^2 + 1 DIRECTLY (s = (d-c)/0.02) via a
    cancellation-safe split-quadratic row basis: per field 9 bf16 rows
    {hilo3(2500 d^2) x coeff 1, hilo3(100 d) x split coeffs of cp,
    cross-level products} plus 3 shared ones-rows carrying hilo3(cp^2+1),
    where cp = (31.5-k)/0.21.  Max relative error on r = 1/X is ~0.4%.
  - Elementwise then needs only a RECIPROCAL: DVE 1-pass custom Newton
    (LORENTZRM fuses the iy weight via broadcast Src1) and ScalarE 1-pass
    Reciprocal activation for the plain (w,v) pair.
  - G accumulates per chunk: G += RTW_c^T PL_c giving quadrants
    [G_uw | G_uv; junk | G_wv]; normalization cancels all global scales.
"""

import numpy as np

P = 128
NCHUNK = 512          # 128-pixel chunks per image
NPIX = 65536
D = 64
EPS = 1e-6
N_CORES = 8
CB = 4                # chunks per block
NBLK = NCHUNK // CB   # 128
NBT = 8               # blocks per batched transpose
CH0, CH1 = -0.23549792, 2.0017324   # Chebyshev recip seed constants
K_SC = 50.0           # s = 50*d + cp

# plain (w,v) Lorentzian pair routing: every PLAIN_DVE_MOD-th group on DVE
# (1-pass custom), rest on ScalarE 1-pass Reciprocal. 0 disables DVE routing.
PLAIN_DVE_MOD = 0

# slot layout (32 slots x CB=4 chunks; TF row = 4*s + c)
_F_SLOTS = {0: 0, 1: 9, 2: 18}       # base slot per field
_S_A = (0, 1, 2)                     # hilo3(2500 d^2), coeff 1
_S_BH = (3, 4, 5)                    # bf16(100 d) x {cph, cpl, cpl2}
_S_BL = (6, 7)                       # lo(100 d)   x {cph, cpl}
_S_BL2 = (8,)                        # lo2(100 d)  x {cph}
_S_ONES = (27, 28, 29)               # 1.0 x hilo3(cp^2 + 1)
_S_ZERO = (30, 31)

_CACHE = {}


def _bf16(a):
    a = np.asarray(a, np.float32)
    x32 = a.view(np.uint32)
    r = ((x32 + 0x7fff + ((x32 >> 16) & 1)) & 0xFFFF0000).astype(np.uint32)
    return r.view(np.float32)


def _register_dve_ops():
    import concourse.dve_ops as dve_ops
    if "LORENTZR" in dve_ops._SUB_OPCODE_FOR_NAME:
        ops = {o.name: o for o in dve_ops.OPS}
        return ops["LORENTZR"], ops["LORENTZRM"]
    from concourse.dve_spec import Spec, Src0, Src1, C0, C1, AluOp, Bin
    from concourse.dve_spec import lower, _has_src1
    from concourse.dve_uop import DveOpSpec

    def _mk(name, body, ref):
        spec = Spec(body=body, reference=ref)
        row = dve_ops._CUSTOM_DVE_ROW_BASE + len(dve_ops.OPS)
        shas = {}
        for ver in ("v3", "v4"):
            tmp = DveOpSpec(name=name, opcode=row,
                            uops=lower(spec, ver=ver), rd1_en=_has_src1(spec))
            shas[ver] = tmp.sha(ver)
        op = dve_ops.DveOp(name, spec, subdim=False, uops_sha=shas)
        dve_ops.OPS.append(op)
        dve_ops.CUSTOM_DVE_SPECS[name] = spec
        dve_ops._SUB_OPCODE_FOR_NAME[name] = row
        return op

    def _recip1nr(xx):
        nxx = (~xx.view(np.int32)).view(np.float32)
        y0 = nxx * np.float32(CH0)
        return y0 * (np.float32(CH1) - xx * y0)

    # plain: out = recip(in0)   (in0 = s^2+1 from the matmul)
    n1 = Bin(AluOp.BITWISE_NOT, Src0, Src0)
    y1 = n1 * C0
    op_plain = _mk(
        "LORENTZR", y1 * (C1 - Src0 * y1),
        lambda in0, in1, s0, s1, imm2: _recip1nr(in0.astype(np.float32)))

    # weighted: out = Src1 * recip(in0)   (Src1 = broadcast iy column)
    n2 = Bin(AluOp.BITWISE_NOT, Src0, Src0)
    y2 = n2 * C0
    op_m = _mk(
        "LORENTZRM", (y2 * (C1 - Src0 * y2)) * Src1,
        lambda in0, in1, s0, s1, imm2:
            in1.astype(np.float32) * _recip1nr(in0.astype(np.float32)))
    return op_plain, op_m


def _build_cc():
    """cc coefficient tensor [128, 2, 384] fp32 (cast to bf16 on chip).

    Row = 4*slot + chunk-in-block. Pair m covers chunks (2m, 2m+1); its 384
    columns are col = j*192 + f*64 + k, producing
    X = s^2 + 1 = 2500 d_f^2 + cp_k*(100 d_f) + (cp_k^2 + 1).
    """
    cp = (31.5 - np.arange(D, dtype=np.float64)) / 0.21
    cph = _bf16(cp)
    cpl = _bf16(cp - np.float64(cph))
    cpl2 = _bf16(cp - np.float64(cph) - np.float64(cpl))
    c0 = cp * cp + 1.0
    c0h = _bf16(c0)
    c0l = _bf16(c0 - np.float64(c0h))
    c0l2 = _bf16(c0 - np.float64(c0h) - np.float64(c0l))

    cc = np.zeros((128, 2, 384), np.float32)
    for m in range(2):
        for j in range(2):
            c = 2 * m + j
            for f in range(3):
                o = j * 192 + f * 64
                b = _F_SLOTS[f]
                for s in _S_A:
                    cc[4 * (b + s) + c, m, o:o + 64] = 1.0
                for s, cf in zip(_S_BH, (cph, cpl, cpl2)):
                    cc[4 * (b + s) + c, m, o:o + 64] = cf
                for s, cf in zip(_S_BL, (cph, cpl)):
                    cc[4 * (b + s) + c, m, o:o + 64] = cf
                cc[4 * (b + _S_BL2[0]) + c, m, o:o + 64] = cph
            # shared ones rows span all 3 fields' columns
            for s, cf in zip(_S_ONES, (c0h, c0l, c0l2)):
                for f in range(3):
                    o = j * 192 + f * 64
                    cc[4 * s + c, m, o:o + 64] = cf
    return cc


def _build_module():
    import concourse.bass as bass
    import concourse.mybir as mybir
    from concourse import bacc
    from concourse.tile import TileContext
    import contextlib

    OP_PLAIN, OP_M = _register_dve_ops()

    f32 = mybir.dt.float32
    bf16 = mybir.dt.bfloat16
    AF = mybir.ActivationFunctionType
    ALU = mybir.AluOpType
    AX = mybir.AxisListType

    nc = bacc.Bacc("TRN2", target_bir_lowering=False, debug=False,
                   num_devices=N_CORES)

    x_dram = nc.dram_tensor("x_img", (3, NPIX), f32, kind="ExternalInput")
    y_dram = nc.dram_tensor("y_img", (3, NPIX), f32, kind="ExternalInput")
    h_dram = nc.dram_tensor("h_out", (1, 1), f32, kind="ExternalOutput")
    cc_np = _build_cc()
    cc_dram = nc.inline_tensor(cc_np, name="cc_const")

    for val in (float(EPS), 0.0):
        t = nc.alloc_sbuf_tensor(f"const-{val}", [128, 1], f32)
        nc.gpsimd.memset(t.ap(), val)
        nc.const_aps.aps[(f32, float(val))] = t.ap()
    nc.all_engine_barrier()

    def direct_recip(out_ap, in_ap, bias):
        imm = lambda v: mybir.ImmediateValue(dtype=f32, value=float(v))
        nc.scalar.add_instruction(
            mybir.InstActivation(
                name=nc.get_next_instruction_name(),
                func=AF.Reciprocal,
                ins=[nc.scalar.lower_ap(in_ap), imm(bias), imm(1.0),
                     imm(0.0)],
                outs=[nc.scalar.lower_ap(out_ap)],
            ))

    with TileContext(nc) as tc:
        with contextlib.ExitStack() as ctx:
            singles = ctx.enter_context(tc.tile_pool(name="singles", bufs=1))
            s1 = ctx.enter_context(tc.tile_pool(name="s1", bufs=1))
            tfp = ctx.enter_context(tc.tile_pool(name="tfp", bufs=3))
            plp = ctx.enter_context(tc.tile_pool(name="plp", bufs=6))
            rtp = ctx.enter_context(tc.tile_pool(name="rtp", bufs=6))
            fin = ctx.enter_context(tc.tile_pool(name="fin", bufs=2))
            gpool = ctx.enter_context(
                tc.tile_pool(name="gpool", bufs=1, space="PSUM"))
            apool = ctx.enter_context(
                tc.tile_pool(name="apool", bufs=3, space="PSUM"))

            cc32 = singles.tile([128, 2, 384], f32, tag="cc32")
            nc.gpsimd.dma_start(out=cc32[:], in_=cc_dram.ap())
            ccb = singles.tile([128, 2, 384], bf16, tag="ccb")
            nc.vector.tensor_copy(out=ccb[:].rearrange("p a b -> p (a b)"),
                                  in_=cc32[:].rearrange("p a b -> p (a b)"))

            # ---------------- stage 1: per-pixel features (both images) -----
            feats = []
            xy = [x_dram, y_dram]
            for ui in range(2):
                X = s1.tile([128, 3, NCHUNK], f32, tag=f"X{ui}")
                src = xy[ui].ap().rearrange("c (p t) -> c p t", p=128)
                for ch in range(3):
                    nc.gpsimd.dma_start(out=X[:, ch, :], in_=src[ch])
                L = s1.tile([128, 3, NCHUNK], f32, tag=f"L{ui}")
                SQ = s1.tile([128, 3, NCHUNK], f32, tag=f"SQ{ui}")
                for ch in range(3):
                    nc.scalar.activation(out=L[:, ch, :], in_=X[:, ch, :],
                                         func=AF.Ln, bias=float(EPS))
                    nc.scalar.activation(out=SQ[:, ch, :], in_=X[:, ch, :],
                                         func=AF.Square, bias=float(EPS))
                SS = s1.tile([128, NCHUNK], f32, tag=f"SS{ui}")
                nc.gpsimd.tensor_add(SS[:], SQ[:, 0, :], SQ[:, 1, :])
                nc.gpsimd.tensor_add(SS[:], SS[:], SQ[:, 2, :])
                IY = s1.tile([128, NCHUNK], f32, tag=f"IY{ui}")
                nc.scalar.activation(out=IY[:], in_=SS[:], func=AF.Sqrt)

                U = s1.tile([128, NCHUNK], f32, tag=f"U{ui}")
                W = s1.tile([128, NCHUNK], f32, tag=f"W{ui}")
                V = s1.tile([128, NCHUNK], f32, tag=f"V{ui}")
                nc.vector.tensor_sub(U[:], L[:, 0, :], L[:, 1, :])
                nc.vector.tensor_sub(W[:], L[:, 1, :], L[:, 2, :])
                nc.vector.tensor_sub(V[:], L[:, 0, :], L[:, 2, :])

                FEAT = s1.tile([128, NBLK, 32, CB], bf16, tag=f"FEAT{ui}")
                for s in _S_ONES:
                    nc.gpsimd.memset(FEAT[:, :, s, :], 1.0)
                for s in _S_ZERO:
                    nc.gpsimd.memset(FEAT[:, :, s, :], 0.0)

                def slot(sl):
                    return FEAT[:, :, sl, :]

                def r512(t):
                    return t[:].rearrange("p (a b) -> p a b", a=NBLK)

                SD = s1.tile([128, NCHUNK], f32, tag=f"SD{ui}")
                QA = s1.tile([128, NCHUNK], f32, tag=f"QA{ui}")
                BD = s1.tile([128, NCHUNK], f32, tag=f"BD{ui}")
                PH = s1.tile([128, NCHUNK], bf16, tag=f"PH{ui}")
                PH2 = s1.tile([128, NCHUNK], bf16, tag=f"PH2{ui}")
                TMP = s1.tile([128, NCHUNK], f32, tag=f"TMP{ui}")
                TMP2 = s1.tile([128, NCHUNK], f32, tag=f"TMP2{ui}")
                for f, dd in ((0, U), (1, W), (2, V)):
                    b = _F_SLOTS[f]
                    # chain engine: alternate Pool / DVE to shorten the span
                    eng = nc.gpsimd if f != 1 else nc.vector
                    # SD = 50*d ; QA = 2500*d^2 ; BD = 100*d
                    nc.vector.tensor_scalar_mul(out=SD[:], in0=dd[:],
                                                scalar1=K_SC)
                    nc.scalar.activation(out=QA[:], in_=SD[:],
                                         func=AF.Square)
                    nc.vector.tensor_scalar_mul(out=BD[:], in0=SD[:],
                                                scalar1=2.0)
                    # hilo3 of QA -> slots A1..A3
                    nc.vector.tensor_copy(out=PH[:], in_=QA[:])
                    nc.sync.dma_start(out=slot(b + _S_A[0]), in_=r512(PH))
                    eng.tensor_tensor(
                        out=slot(b + _S_A[1]), in0=r512(QA),
                        in1=r512(PH), op=ALU.subtract)
                    eng.tensor_tensor(out=r512(TMP), in0=r512(PH),
                                      in1=slot(b + _S_A[1]), op=ALU.add)
                    eng.tensor_tensor(
                        out=slot(b + _S_A[2]), in0=r512(QA),
                        in1=r512(TMP), op=ALU.subtract)
                    # hilo3 of BD -> Bh (x3 slots), Bl (x2), Bl2
                    nc.vector.tensor_copy(out=PH2[:], in_=BD[:])
                    for s in _S_BH:
                        nc.sync.dma_start(out=slot(b + s), in_=r512(PH2))
                    eng.tensor_tensor(
                        out=slot(b + _S_BL[0]), in0=r512(BD),
                        in1=r512(PH2), op=ALU.subtract)
                    nc.vector.tensor_copy(out=slot(b + _S_BL[1]),
                                          in_=slot(b + _S_BL[0]))
                    eng.tensor_tensor(out=r512(TMP2), in0=r512(PH2),
                                      in1=slot(b + _S_BL[0]), op=ALU.add)
                    eng.tensor_tensor(
                        out=slot(b + _S_BL2[0]), in0=r512(BD),
                        in1=r512(TMP2), op=ALU.subtract)
                feats.append((FEAT, IY))

            # ---------------- stage 2: blocks -------------------------------
            units = []
            for ui in range(2):
                FEAT, IY = feats[ui]
                G = gpool.tile([128, 128], f32, tag=f"G{ui}")
                units.append(G)
                pending = None
                TF = None

                def flush(pend):
                    if pend is None:
                        return
                    RTWp, PLp, c0_, nchp = pend
                    for c in range(nchp):
                        ch_g = c0_ + c
                        nc.tensor.matmul(
                            out=G[:],
                            lhsT=RTWp[:, c, :],
                            rhs=PLp[:, c, :],
                            start=(ch_g == 0), stop=(ch_g == NCHUNK - 1),
                            skip_group_check=True)

                chunk = 0
                for b in range(NBLK):
                    i = b % NBT
                    if i == 0:
                        TF = tfp.tile([128, NBT, 128], bf16, tag="TF")
                        nc.sync.dma_start_transpose(
                            out=TF[:],
                            in_=FEAT[:, b:b + NBT].rearrange(
                                "p a s c -> p (a s c)"))
                    nch = CB
                    A = apool.tile([128, 4, 256], f32, tag="A")
                    PL = plp.tile([128, 4, 128], bf16, tag="PL")
                    RTW = rtp.tile([128, 4, 128], bf16, tag="RTW")
                    for j in range(2):
                        nc.tensor.matmul(
                            out=A[:, 2 * j:2 * j + 2, 0:192],
                            lhsT=TF[:, i, :], rhs=ccb[:, j, :],
                            start=True, stop=True)
                    flush(pending)

                    # weighted pair: RTW = [iy*Pu | iy*Pw], one DVE pass
                    nc.vector._custom_dve(
                        OP_M, out=RTW[:, 0:nch, :],
                        in0=A[:, 0:nch, 0:128],
                        in1=IY[:, chunk:chunk + nch].unsqueeze(2)
                            .broadcast_to([128, nch, 128]),
                        s0=CH0, s1=CH1)
                    # plain pair (w, v): 1-pass Reciprocal
                    if PLAIN_DVE_MOD and (b % PLAIN_DVE_MOD
                                          == PLAIN_DVE_MOD - 1):
                        nc.vector._custom_dve(
                            OP_PLAIN, out=PL[:, 0:nch, :],
                            in0=A[:, 0:nch, 64:192],
                            s0=CH0, s1=CH1)
                    else:
                        direct_recip(PL[:, 0:nch, :],
                                     A[:, 0:nch, 64:192], 0.0)

                    pending = (RTW, PL, chunk, nch)
                    chunk += nch
                flush(pending)

            # ---------------- stage 3: normalize + Hellinger ----------------
            SQs = []
            for ui in range(2):
                G = units[ui]
                red = fin.tile([128, 1], f32, tag=f"red{ui}")
                nc.vector.tensor_reduce(out=red[0:64, :], in_=G[0:64, :],
                                        axis=AX.X, op=ALU.add)
                nc.vector.tensor_reduce(out=red[64:128, :],
                                        in_=G[64:128, 64:128],
                                        axis=AX.X, op=ALU.add)
                tot = fin.tile([1, 1], f32, tag=f"tot{ui}")
                nc.gpsimd.tensor_reduce(out=tot[:], in_=red[:], axis=AX.C,
                                        op=ALU.add)
                inv = fin.tile([1, 1], f32, tag=f"inv{ui}")
                nc.vector.reciprocal(out=inv[:], in_=tot[:])
                invb = fin.tile([128, 1], f32, tag=f"invb{ui}")
                nc.gpsimd.partition_broadcast(invb[:], inv[:])
                SQt = fin.tile([128, 128], f32, tag=f"SQt{ui}")
                nc.scalar.activation(out=SQt[0:64, :], in_=G[0:64, :],
                                     func=AF.Sqrt, scale=invb[0:64, 0:1])
                nc.scalar.activation(out=SQt[64:128, 64:128],
                                     in_=G[64:128, 64:128],
                                     func=AF.Sqrt, scale=invb[64:128, 0:1])
                SQs.append(SQt)

            DF = fin.tile([128, 128], f32, tag="DF")
            nc.vector.tensor_sub(DF[0:64, :], SQs[1][0:64, :],
                                 SQs[0][0:64, :])
            nc.vector.tensor_sub(DF[64:128, 64:128],
                                 SQs[1][64:128, 64:128],
                                 SQs[0][64:128, 64:128])
            SC2 = fin.tile([128, 128], f32, tag="SC2")
            acc = fin.tile([128, 1], f32, tag="acc")
            nc.scalar.activation(out=SC2[0:64, :], in_=DF[0:64, :],
                                 func=AF.Square, accum_out=acc[0:64, :])
            nc.scalar.activation(out=SC2[64:128, 64:128],
                                 in_=DF[64:128, 64:128],
                                 func=AF.Square, accum_out=acc[64:128, :])
            htot = fin.tile([1, 1], f32, tag="htot")
            nc.gpsimd.tensor_reduce(out=htot[:], in_=acc[:], axis=AX.C,
                                    op=ALU.add)
            hres = fin.tile([1, 1], f32, tag="hres")
            nc.scalar.activation(out=hres[:], in_=htot[:], func=AF.Sqrt,
                                 scale=0.5)
            nc.sync.dma_start(out=h_dram.ap(), in_=hres[:])

    nc.finalize()
    return nc


def _get_module():
    if "nc" not in _CACHE:
        _CACHE["nc"] = _build_module()
    return _CACHE["nc"]


def _run(x, y, trace=False):
    from concourse.bass_utils import run_bass_kernel_spmd
    nc = _get_module()
    x = np.ascontiguousarray(np.asarray(x, np.float32).reshape(8, 3, NPIX))
    y = np.ascontiguousarray(np.asarray(y, np.float32).reshape(8, 3, NPIX))
    in_maps = [{"x_img": x[i], "y_img": y[i]} for i in range(N_CORES)]
    res = run_bass_kernel_spmd(nc, in_maps, core_ids=list(range(N_CORES)),
                               trace=trace)
    hs = np.array([res.results[i]["h_out"].reshape(-1)[0]
                   for i in range(N_CORES)], np.float64)
    return hs, res


def kernel(x, y):
    hs, _ = _run(x, y)
    return np.float32(hs.mean())


# revision 20
# speedup vs baseline: 1.3921x; 1.3921x over previous
"""Trainium2 Bass kernel for nn_ColorHistogramMatchingLoss (v6).

Data-parallel over batch: core i processes image pair (x[i], y[i]) and emits
the per-image Hellinger distance; the host averages 8 scalars.

v6 reformulation (validated in numpy):
  - The three histograms reduce (via flip/transpose invariance of the loss)
    to G_uv = (iy Ru)^T Rv, G_uw = (iy Ru)^T Rw, G_wv = (iy Rw)^T Rv over the
    log-ratio fields u=lr-lg, w=lg-lb, v=lr-lb, and the weighted RBF column
    is iy * plain Lorentzian of the same field.
  - The PE matmul produces X = s^2 + 1 DIRECTLY (s = (d-c)/0.02) via a
    cancellation-safe split-quadratic row basis: per field 9 bf16 rows
    {hilo3(2500 d^2) x coeff 1, hilo3(100 d) x split coeffs of cp,
    cross-level products} plus 3 shared ones-rows carrying hilo3(cp^2+1),
    where cp = (31.5-k)/0.21.  Max relative error on r = 1/X is ~0.4%.
  - Elementwise then needs only a RECIPROCAL: DVE 1-pass custom Newton
    (LORENTZRM fuses the iy weight via broadcast Src1) and ScalarE 1-pass
    Reciprocal activation for the plain (w,v) pair.
  - G accumulates per chunk: G += RTW_c^T PL_c giving quadrants
    [G_uw | G_uv; junk | G_wv]; normalization cancels all global scales.
"""

import numpy as np

P = 128
NCHUNK = 512          # 128-pixel chunks per image
NPIX = 65536
D = 64
EPS = 1e-6
N_CORES = 8
CB = 4                # chunks per block
NBLK = NCHUNK // CB   # 128
NBT = 8               # blocks per batched transpose
CH0, CH1 = -0.23549792, 2.0017324   # Chebyshev recip seed constants
K_SC = 50.0           # s = 50*d + cp

# plain (w,v) Lorentzian pair routing: every PLAIN_DVE_MOD-th group on DVE
# (1-pass custom), rest on ScalarE 1-pass Reciprocal. 0 disables DVE routing.
PLAIN_DVE_MOD = 0

# slot layout (32 slots x CB=4 chunks; TF row = 4*s + c)
_F_SLOTS = {0: 0, 1: 9, 2: 18}       # base slot per field
_S_A = (0, 1, 2)                     # hilo3(2500 d^2), coeff 1
_S_BH = (3, 4, 5)                    # bf16(100 d) x {cph, cpl, cpl2}
_S_BL = (6, 7)                       # lo(100 d)   x {cph, cpl}
_S_BL2 = (8,)                        # lo2(100 d)  x {cph}
_S_ONES = (27, 28, 29)               # 1.0 x hilo3(cp^2 + 1)
_S_ZERO = (30, 31)

_CACHE = {}


def _bf16(a):
    a = np.asarray(a, np.float32)
    x32 = a.view(np.uint32)
    r = ((x32 + 0x7fff + ((x32 >> 16) & 1)) & 0xFFFF0000).astype(np.uint32)
    return r.view(np.float32)


def _register_dve_ops():
    import concourse.dve_ops as dve_ops
    if "LORENTZR" in dve_ops._SUB_OPCODE_FOR_NAME:
        ops = {o.name: o for o in dve_ops.OPS}
        return ops["LORENTZR"], ops["LORENTZRM"]
    from concourse.dve_spec import Spec, Src0, Src1, C0, C1, AluOp, Bin
    from concourse.dve_spec import lower, _has_src1
    from concourse.dve_uop import DveOpSpec

    def _mk(name, body, ref):
        spec = Spec(body=body, reference=ref)
        row = dve_ops._CUSTOM_DVE_ROW_BASE + len(dve_ops.OPS)
        shas = {}
        for ver in ("v3", "v4"):
            tmp = DveOpSpec(name=name, opcode=row,
                            uops=lower(spec, ver=ver), rd1_en=_has_src1(spec))
            shas[ver] = tmp.sha(ver)
        op = dve_ops.DveOp(name, spec, subdim=False, uops_sha=shas)
        dve_ops.OPS.append(op)
        dve_ops.CUSTOM_DVE_SPECS[name] = spec
        dve_ops._SUB_OPCODE_FOR_NAME[name] = row
        return op

    def _recip1nr(xx):
        nxx = (~xx.view(np.int32)).view(np.float32)
        y0 = nxx * np.float32(CH0)
        return y0 * (np.float32(CH1) - xx * y0)

    # plain: out = recip(in0)   (in0 = s^2+1 from the matmul)
    n1 = Bin(AluOp.BITWISE_NOT, Src0, Src0)
    y1 = n1 * C0
    op_plain = _mk(
        "LORENTZR", y1 * (C1 - Src0 * y1),
        lambda in0, in1, s0, s1, imm2: _recip1nr(in0.astype(np.float32)))

    # weighted: out = Src1 * recip(in0)   (Src1 = broadcast iy column)
    n2 = Bin(AluOp.BITWISE_NOT, Src0, Src0)
    y2 = n2 * C0
    op_m = _mk(
        "LORENTZRM", (y2 * (C1 - Src0 * y2)) * Src1,
        lambda in0, in1, s0, s1, imm2:
            in1.astype(np.float32) * _recip1nr(in0.astype(np.float32)))
    return op_plain, op_m


def _build_cc():
    """cc coefficient tensor [128, 2, 384] fp32 (cast to bf16 on chip).

    Row = 4*slot + chunk-in-block. Pair m covers chunks (2m, 2m+1); its 384
    columns are col = j*192 + f*64 + k, producing
    X = s^2 + 1 = 2500 d_f^2 + cp_k*(100 d_f) + (cp_k^2 + 1).
    """
    cp = (31.5 - np.arange(D, dtype=np.float64)) / 0.21
    cph = _bf16(cp)
    cpl = _bf16(cp - np.float64(cph))
    cpl2 = _bf16(cp - np.float64(cph) - np.float64(cpl))
    c0 = cp * cp + 1.0
    c0h = _bf16(c0)
    c0l = _bf16(c0 - np.float64(c0h))
    c0l2 = _bf16(c0 - np.float64(c0h) - np.float64(c0l))

    cc = np.zeros((128, 2, 384), np.float32)
    for m in range(2):
        for j in range(2):
            c = 2 * m + j
            for f in range(3):
                o = j * 192 + f * 64
                b = _F_SLOTS[f]
                for s in _S_A:
                    cc[4 * (b + s) + c, m, o:o + 64] = 1.0
                for s, cf in zip(_S_BH, (cph, cpl, cpl2)):
                    cc[4 * (b + s) + c, m, o:o + 64] = cf
                for s, cf in zip(_S_BL, (cph, cpl)):
                    cc[4 * (b + s) + c, m, o:o + 64] = cf
                cc[4 * (b + _S_BL2[0]) + c, m, o:o + 64] = cph
            # shared ones rows span all 3 fields' columns
            for s, cf in zip(_S_ONES, (c0h, c0l, c0l2)):
                for f in range(3):
                    o = j * 192 + f * 64
                    cc[4 * s + c, m, o:o + 64] = cf
    return cc


def _build_module():
    import concourse.bass as bass
    import concourse.mybir as mybir
    from concourse import bacc
    from concourse.tile import TileContext
    import contextlib

    OP_PLAIN, OP_M = _register_dve_ops()

    f32 = mybir.dt.float32
    bf16 = mybir.dt.bfloat16
    AF = mybir.ActivationFunctionType
    ALU = mybir.AluOpType
    AX = mybir.AxisListType

    nc = bacc.Bacc("TRN2", target_bir_lowering=False, debug=False,
                   num_devices=N_CORES)

    x_dram = nc.dram_tensor("x_img", (3, NPIX), f32, kind="ExternalInput")
    y_dram = nc.dram_tensor("y_img", (3, NPIX), f32, kind="ExternalInput")
    h_dram = nc.dram_tensor("h_out", (1, 1), f32, kind="ExternalOutput")
    cc_np = _build_cc()
    cc_dram = nc.inline_tensor(cc_np, name="cc_const")

    for val in (float(EPS), 0.0):
        t = nc.alloc_sbuf_tensor(f"const-{val}", [128, 1], f32)
        nc.gpsimd.memset(t.ap(), val)
        nc.const_aps.aps[(f32, float(val))] = t.ap()
    nc.all_engine_barrier()

    def direct_recip(out_ap, in_ap, bias):
        imm = lambda v: mybir.ImmediateValue(dtype=f32, value=float(v))
        nc.scalar.add_instruction(
            mybir.InstActivation(
                name=nc.get_next_instruction_name(),
                func=AF.Reciprocal,
                ins=[nc.scalar.lower_ap(in_ap), imm(bias), imm(1.0),
                     imm(0.0)],
                outs=[nc.scalar.lower_ap(out_ap)],
            ))

    with TileContext(nc) as tc:
        with contextlib.ExitStack() as ctx:
            singles = ctx.enter_context(tc.tile_pool(name="singles", bufs=1))
            s1 = ctx.enter_context(tc.tile_pool(name="s1", bufs=1))
            tfp = ctx.enter_context(tc.tile_pool(name="tfp", bufs=3))
            plp = ctx.enter_context(tc.tile_pool(name="plp", bufs=6))
            rtp = ctx.enter_context(tc.tile_pool(name="rtp", bufs=6))
            fin = ctx.enter_context(tc.tile_pool(name="fin", bufs=2))
            gpool = ctx.enter_context(
                tc.tile_pool(name="gpool", bufs=1, space="PSUM"))
            apool = ctx.enter_context(
                tc.tile_pool(name="apool", bufs=3, space="PSUM"))

            cc32 = singles.tile([128, 2, 384], f32, tag="cc32")
            nc.gpsimd.dma_start(out=cc32[:], in_=cc_dram.ap())
            ccb = singles.tile([128, 2, 384], bf16, tag="ccb")
            nc.vector.tensor_copy(out=ccb[:].rearrange("p a b -> p (a b)"),
                                  in_=cc32[:].rearrange("p a b -> p (a b)"))

            # ---------------- stage 1: per-pixel features (both images) -----
            feats = []
            xy = [x_dram, y_dram]
            for ui in range(2):
                X = s1.tile([128, 3, NCHUNK], f32, tag=f"X{ui}")
                src = xy[ui].ap().rearrange("c (p t) -> c p t", p=128)
                for ch in range(3):
                    nc.gpsimd.dma_start(out=X[:, ch, :], in_=src[ch])
                L = s1.tile([128, 3, NCHUNK], f32, tag=f"L{ui}")
                SQ = s1.tile([128, 3, NCHUNK], f32, tag=f"SQ{ui}")
                for ch in range(3):
                    nc.scalar.activation(out=L[:, ch, :], in_=X[:, ch, :],
                                         func=AF.Ln, bias=float(EPS))
                    nc.scalar.activation(out=SQ[:, ch, :], in_=X[:, ch, :],
                                         func=AF.Square, bias=float(EPS))
                SS = s1.tile([128, NCHUNK], f32, tag=f"SS{ui}")
                nc.gpsimd.tensor_add(SS[:], SQ[:, 0, :], SQ[:, 1, :])
                nc.gpsimd.tensor_add(SS[:], SS[:], SQ[:, 2, :])
                IY = s1.tile([128, NCHUNK], f32, tag=f"IY{ui}")
                nc.scalar.activation(out=IY[:], in_=SS[:], func=AF.Sqrt)

                U = s1.tile([128, NCHUNK], f32, tag=f"U{ui}")
                W = s1.tile([128, NCHUNK], f32, tag=f"W{ui}")
                V = s1.tile([128, NCHUNK], f32, tag=f"V{ui}")
                nc.vector.tensor_sub(U[:], L[:, 0, :], L[:, 1, :])
                nc.vector.tensor_sub(W[:], L[:, 1, :], L[:, 2, :])
                nc.vector.tensor_sub(V[:], L[:, 0, :], L[:, 2, :])

                FEAT = s1.tile([128, NBLK, 32, CB], bf16, tag=f"FEAT{ui}")
                for s in _S_ONES:
                    nc.gpsimd.memset(FEAT[:, :, s, :], 1.0)
                for s in _S_ZERO:
                    nc.gpsimd.memset(FEAT[:, :, s, :], 0.0)

                def slot(sl):
                    return FEAT[:, :, sl, :]

                def r512(t):
                    return t[:].rearrange("p (a b) -> p a b", a=NBLK)

                SD = s1.tile([128, NCHUNK], f32, tag=f"SD{ui}")
                QA = s1.tile([128, NCHUNK], f32, tag=f"QA{ui}")
                BD = s1.tile([128, NCHUNK], f32, tag=f"BD{ui}")
                TMP = s1.tile([128, NCHUNK], f32, tag=f"TMP{ui}")
                TMP2 = s1.tile([128, NCHUNK], f32, tag=f"TMP2{ui}")
                for f, dd in ((0, U), (1, W), (2, V)):
                    b = _F_SLOTS[f]
                    # chain engine: alternate Pool / DVE to shorten the span
                    eng = nc.gpsimd if f != 1 else nc.vector
                    # SD = 50*d ; QA = 2500*d^2 ; BD = 100*d
                    nc.vector.tensor_scalar_mul(out=SD[:], in0=dd[:],
                                                scalar1=K_SC)
                    nc.scalar.activation(out=QA[:], in_=SD[:],
                                         func=AF.Square)
                    nc.vector.tensor_scalar_mul(out=BD[:], in0=SD[:],
                                                scalar1=2.0)
                    # hilo3 of QA -> slots A1..A3 (hi lands in the slot via a
                    # strided cast copy; lo terms via TT subtracts)
                    nc.vector.tensor_copy(out=slot(b + _S_A[0]),
                                          in_=r512(QA))
                    eng.tensor_tensor(
                        out=slot(b + _S_A[1]), in0=r512(QA),
                        in1=slot(b + _S_A[0]), op=ALU.subtract)
                    eng.tensor_tensor(out=r512(TMP), in0=slot(b + _S_A[0]),
                                      in1=slot(b + _S_A[1]), op=ALU.add)
                    eng.tensor_tensor(
                        out=slot(b + _S_A[2]), in0=r512(QA),
                        in1=r512(TMP), op=ALU.subtract)
                    # hilo3 of BD -> Bh (x3 slots), Bl (x2), Bl2
                    nc.vector.tensor_copy(out=slot(b + _S_BH[0]),
                                          in_=r512(BD))
                    for s in _S_BH[1:]:
                        nc.vector.tensor_copy(out=slot(b + s),
                                              in_=slot(b + _S_BH[0]))
                    eng.tensor_tensor(
                        out=slot(b + _S_BL[0]), in0=r512(BD),
                        in1=slot(b + _S_BH[0]), op=ALU.subtract)
                    nc.vector.tensor_copy(out=slot(b + _S_BL[1]),
                                          in_=slot(b + _S_BL[0]))
                    eng.tensor_tensor(out=r512(TMP2), in0=slot(b + _S_BH[0]),
                                      in1=slot(b + _S_BL[0]), op=ALU.add)
                    eng.tensor_tensor(
                        out=slot(b + _S_BL2[0]), in0=r512(BD),
                        in1=r512(TMP2), op=ALU.subtract)
                feats.append((FEAT, IY))

            # ---------------- stage 2: blocks -------------------------------
            units = []
            for ui in range(2):
                FEAT, IY = feats[ui]
                G = gpool.tile([128, 128], f32, tag=f"G{ui}")
                units.append(G)
                pending = None
                TF = None

                def flush(pend):
                    if pend is None:
                        return
                    RTWp, PLp, c0_, nchp = pend
                    for c in range(nchp):
                        ch_g = c0_ + c
                        nc.tensor.matmul(
                            out=G[:],
                            lhsT=RTWp[:, c, :],
                            rhs=PLp[:, c, :],
                            start=(ch_g == 0), stop=(ch_g == NCHUNK - 1),
                            skip_group_check=True)

                chunk = 0
                for b in range(NBLK):
                    i = b % NBT
                    if i == 0:
                        TF = tfp.tile([128, NBT, 128], bf16, tag="TF")
                        nc.sync.dma_start_transpose(
                            out=TF[:],
                            in_=FEAT[:, b:b + NBT].rearrange(
                                "p a s c -> p (a s c)"))
                    nch = CB
                    A = apool.tile([128, 4, 256], f32, tag="A")
                    PL = plp.tile([128, 4, 128], bf16, tag="PL")
                    RTW = rtp.tile([128, 4, 128], bf16, tag="RTW")
                    for j in range(2):
                        nc.tensor.matmul(
                            out=A[:, 2 * j:2 * j + 2, 0:192],
                            lhsT=TF[:, i, :], rhs=ccb[:, j, :],
                            start=True, stop=True)
                    flush(pending)

                    # weighted pair: RTW = [iy*Pu | iy*Pw], one DVE pass
                    nc.vector._custom_dve(
                        OP_M, out=RTW[:, 0:nch, :],
                        in0=A[:, 0:nch, 0:128],
                        in1=IY[:, chunk:chunk + nch].unsqueeze(2)
                            .broadcast_to([128, nch, 128]),
                        s0=CH0, s1=CH1)
                    # plain pair (w, v): 1-pass Reciprocal
                    if PLAIN_DVE_MOD and (b % PLAIN_DVE_MOD
                                          == PLAIN_DVE_MOD - 1):
                        nc.vector._custom_dve(
                            OP_PLAIN, out=PL[:, 0:nch, :],
                            in0=A[:, 0:nch, 64:192],
                            s0=CH0, s1=CH1)
                    else:
                        direct_recip(PL[:, 0:nch, :],
                                     A[:, 0:nch, 64:192], 0.0)

                    pending = (RTW, PL, chunk, nch)
                    chunk += nch
                flush(pending)

            # ---------------- stage 3: normalize + Hellinger ----------------
            SQs = []
            for ui in range(2):
                G = units[ui]
                red = fin.tile([128, 1], f32, tag=f"red{ui}")
                nc.vector.tensor_reduce(out=red[0:64, :], in_=G[0:64, :],
                                        axis=AX.X, op=ALU.add)
                nc.vector.tensor_reduce(out=red[64:128, :],
                                        in_=G[64:128, 64:128],
                                        axis=AX.X, op=ALU.add)
                tot = fin.tile([1, 1], f32, tag=f"tot{ui}")
                nc.gpsimd.tensor_reduce(out=tot[:], in_=red[:], axis=AX.C,
                                        op=ALU.add)
                inv = fin.tile([1, 1], f32, tag=f"inv{ui}")
                nc.vector.reciprocal(out=inv[:], in_=tot[:])
                invb = fin.tile([128, 1], f32, tag=f"invb{ui}")
                nc.gpsimd.partition_broadcast(invb[:], inv[:])
                SQt = fin.tile([128, 128], f32, tag=f"SQt{ui}")
                nc.scalar.activation(out=SQt[0:64, :], in_=G[0:64, :],
                                     func=AF.Sqrt, scale=invb[0:64, 0:1])
                nc.scalar.activation(out=SQt[64:128, 64:128],
                                     in_=G[64:128, 64:128],
                                     func=AF.Sqrt, scale=invb[64:128, 0:1])
                SQs.append(SQt)

            DF = fin.tile([128, 128], f32, tag="DF")
            nc.vector.tensor_sub(DF[0:64, :], SQs[1][0:64, :],
                                 SQs[0][0:64, :])
            nc.vector.tensor_sub(DF[64:128, 64:128],
                                 SQs[1][64:128, 64:128],
                                 SQs[0][64:128, 64:128])
            SC2 = fin.tile([128, 128], f32, tag="SC2")
            acc = fin.tile([128, 1], f32, tag="acc")
            nc.scalar.activation(out=SC2[0:64, :], in_=DF[0:64, :],
                                 func=AF.Square, accum_out=acc[0:64, :])
            nc.scalar.activation(out=SC2[64:128, 64:128],
                                 in_=DF[64:128, 64:128],
                                 func=AF.Square, accum_out=acc[64:128, :])
            htot = fin.tile([1, 1], f32, tag="htot")
            nc.gpsimd.tensor_reduce(out=htot[:], in_=acc[:], axis=AX.C,
                                    op=ALU.add)
            hres = fin.tile([1, 1], f32, tag="hres")
            nc.scalar.activation(out=hres[:], in_=htot[:], func=AF.Sqrt,
                                 scale=0.5)
            nc.sync.dma_start(out=h_dram.ap(), in_=hres[:])

    nc.finalize()
    return nc


def _get_module():
    if "nc" not in _CACHE:
        _CACHE["nc"] = _build_module()
    return _CACHE["nc"]


def _run(x, y, trace=False):
    from concourse.bass_utils import run_bass_kernel_spmd
    nc = _get_module()
    x = np.ascontiguousarray(np.asarray(x, np.float32).reshape(8, 3, NPIX))
    y = np.ascontiguousarray(np.asarray(y, np.float32).reshape(8, 3, NPIX))
    in_maps = [{"x_img": x[i], "y_img": y[i]} for i in range(N_CORES)]
    res = run_bass_kernel_spmd(nc, in_maps, core_ids=list(range(N_CORES)),
                               trace=trace)
    hs = np.array([res.results[i]["h_out"].reshape(-1)[0]
                   for i in range(N_CORES)], np.float64)
    return hs, res


def kernel(x, y):
    hs, _ = _run(x, y)
    return np.float32(hs.mean())


# revision 23
# speedup vs baseline: 1.4511x; 1.0424x over previous
"""Trainium2 Bass kernel for nn_ColorHistogramMatchingLoss (v6).

Data-parallel over batch: core i processes image pair (x[i], y[i]) and emits
the per-image Hellinger distance; the host averages 8 scalars.

v6 reformulation (validated in numpy):
  - The three histograms reduce (via flip/transpose invariance of the loss)
    to G_uv = (iy Ru)^T Rv, G_uw = (iy Ru)^T Rw, G_wv = (iy Rw)^T Rv over the
    log-ratio fields u=lr-lg, w=lg-lb, v=lr-lb, and the weighted RBF column
    is iy * plain Lorentzian of the same field.
  - The PE matmul produces X = s^2 + 1 DIRECTLY (s = (d-c)/0.02) via a
    cancellation-safe split-quadratic row basis: per field 9 bf16 rows
    {hilo3(2500 d^2) x coeff 1, hilo3(100 d) x split coeffs of cp,
    cross-level products} plus 3 shared ones-rows carrying hilo3(cp^2+1),
    where cp = (31.5-k)/0.21.  Max relative error on r = 1/X is ~0.4%.
  - Elementwise then needs only a RECIPROCAL: DVE 1-pass custom Newton
    (LORENTZRM fuses the iy weight via broadcast Src1) and ScalarE 1-pass
    Reciprocal activation for the plain (w,v) pair.
  - G accumulates per chunk: G += RTW_c^T PL_c giving quadrants
    [G_uw | G_uv; junk | G_wv]; normalization cancels all global scales.
"""

import numpy as np

P = 128
NCHUNK = 512          # 128-pixel chunks per image
NPIX = 65536
D = 64
EPS = 1e-6
N_CORES = 8
CB = 4                # chunks per block
NBLK = NCHUNK // CB   # 128
NBT = 8               # blocks per batched transpose
CH0, CH1 = -0.23549792, 2.0017324   # Chebyshev recip seed constants
K_SC = 50.0           # s = 50*d + cp

# plain (w,v) Lorentzian pair routing: every PLAIN_DVE_MOD-th group on DVE
# (1-pass custom), rest on ScalarE 1-pass Reciprocal. 0 disables DVE routing.
PLAIN_DVE_MOD = 0

# slot layout (32 slots x CB=4 chunks; TF row = 4*s + c)
_F_SLOTS = {0: 0, 1: 9, 2: 18}       # base slot per field
_S_A = (0, 1, 2)                     # hilo3(2500 d^2), coeff 1
_S_BH = (3, 4, 5)                    # bf16(100 d) x {cph, cpl, cpl2}
_S_BL = (6, 7)                       # lo(100 d)   x {cph, cpl}
_S_BL2 = (8,)                        # lo2(100 d)  x {cph}
_S_ONES = (27, 28, 29)               # 1.0 x hilo3(cp^2 + 1)
_S_ZERO = (30, 31)

_CACHE = {}


def _bf16(a):
    a = np.asarray(a, np.float32)
    x32 = a.view(np.uint32)
    r = ((x32 + 0x7fff + ((x32 >> 16) & 1)) & 0xFFFF0000).astype(np.uint32)
    return r.view(np.float32)


def _register_dve_ops():
    import concourse.dve_ops as dve_ops
    if "LORENTZR" in dve_ops._SUB_OPCODE_FOR_NAME:
        ops = {o.name: o for o in dve_ops.OPS}
        return ops["LORENTZR"], ops["LORENTZRM"]
    from concourse.dve_spec import Spec, Src0, Src1, C0, C1, AluOp, Bin
    from concourse.dve_spec import lower, _has_src1
    from concourse.dve_uop import DveOpSpec

    def _mk(name, body, ref):
        spec = Spec(body=body, reference=ref)
        row = dve_ops._CUSTOM_DVE_ROW_BASE + len(dve_ops.OPS)
        shas = {}
        for ver in ("v3", "v4"):
            tmp = DveOpSpec(name=name, opcode=row,
                            uops=lower(spec, ver=ver), rd1_en=_has_src1(spec))
            shas[ver] = tmp.sha(ver)
        op = dve_ops.DveOp(name, spec, subdim=False, uops_sha=shas)
        dve_ops.OPS.append(op)
        dve_ops.CUSTOM_DVE_SPECS[name] = spec
        dve_ops._SUB_OPCODE_FOR_NAME[name] = row
        return op

    def _recip1nr(xx):
        nxx = (~xx.view(np.int32)).view(np.float32)
        y0 = nxx * np.float32(CH0)
        return y0 * (np.float32(CH1) - xx * y0)

    # plain: out = recip(in0)   (in0 = s^2+1 from the matmul)
    n1 = Bin(AluOp.BITWISE_NOT, Src0, Src0)
    y1 = n1 * C0
    op_plain = _mk(
        "LORENTZR", y1 * (C1 - Src0 * y1),
        lambda in0, in1, s0, s1, imm2: _recip1nr(in0.astype(np.float32)))

    # weighted: out = Src1 * recip(in0)   (Src1 = broadcast iy column)
    n2 = Bin(AluOp.BITWISE_NOT, Src0, Src0)
    y2 = n2 * C0
    op_m = _mk(
        "LORENTZRM", (y2 * (C1 - Src0 * y2)) * Src1,
        lambda in0, in1, s0, s1, imm2:
            in1.astype(np.float32) * _recip1nr(in0.astype(np.float32)))
    return op_plain, op_m


def _build_cc():
    """cc coefficient tensor [128, 2, 384] fp32 (cast to bf16 on chip).

    Row = 4*slot + chunk-in-block. Pair m covers chunks (2m, 2m+1); its 384
    columns are col = j*192 + f*64 + k, producing
    X = s^2 + 1 = 2500 d_f^2 + cp_k*(100 d_f) + (cp_k^2 + 1).
    """
    cp = (31.5 - np.arange(D, dtype=np.float64)) / 0.21
    cph = _bf16(cp)
    cpl = _bf16(cp - np.float64(cph))
    cpl2 = _bf16(cp - np.float64(cph) - np.float64(cpl))
    c0 = cp * cp + 1.0
    c0h = _bf16(c0)
    c0l = _bf16(c0 - np.float64(c0h))
    c0l2 = _bf16(c0 - np.float64(c0h) - np.float64(c0l))

    cc = np.zeros((128, 2, 384), np.float32)
    for m in range(2):
        for j in range(2):
            c = 2 * m + j
            for f in range(3):
                o = j * 192 + f * 64
                b = _F_SLOTS[f]
                for s in _S_A:
                    cc[4 * (b + s) + c, m, o:o + 64] = 1.0
                for s, cf in zip(_S_BH, (cph, cpl, cpl2)):
                    cc[4 * (b + s) + c, m, o:o + 64] = cf
                for s, cf in zip(_S_BL, (cph, cpl)):
                    cc[4 * (b + s) + c, m, o:o + 64] = cf
                cc[4 * (b + _S_BL2[0]) + c, m, o:o + 64] = cph
            # shared ones rows span all 3 fields' columns
            for s, cf in zip(_S_ONES, (c0h, c0l, c0l2)):
                for f in range(3):
                    o = j * 192 + f * 64
                    cc[4 * s + c, m, o:o + 64] = cf
    return cc


def _build_module():
    import concourse.bass as bass
    import concourse.mybir as mybir
    from concourse import bacc
    from concourse.tile import TileContext
    import contextlib

    OP_PLAIN, OP_M = _register_dve_ops()

    f32 = mybir.dt.float32
    bf16 = mybir.dt.bfloat16
    AF = mybir.ActivationFunctionType
    ALU = mybir.AluOpType
    AX = mybir.AxisListType

    nc = bacc.Bacc("TRN2", target_bir_lowering=False, debug=False,
                   num_devices=N_CORES)

    x_dram = nc.dram_tensor("x_img", (3, NPIX), f32, kind="ExternalInput")
    y_dram = nc.dram_tensor("y_img", (3, NPIX), f32, kind="ExternalInput")
    h_dram = nc.dram_tensor("h_out", (1, 1), f32, kind="ExternalOutput")
    cc_np = _build_cc()
    cc_dram = nc.inline_tensor(cc_np, name="cc_const")

    for val in (float(EPS), 0.0):
        t = nc.alloc_sbuf_tensor(f"const-{val}", [128, 1], f32)
        nc.gpsimd.memset(t.ap(), val)
        nc.const_aps.aps[(f32, float(val))] = t.ap()
    nc.all_engine_barrier()

    def direct_recip(out_ap, in_ap, bias):
        imm = lambda v: mybir.ImmediateValue(dtype=f32, value=float(v))
        nc.scalar.add_instruction(
            mybir.InstActivation(
                name=nc.get_next_instruction_name(),
                func=AF.Reciprocal,
                ins=[nc.scalar.lower_ap(in_ap), imm(bias), imm(1.0),
                     imm(0.0)],
                outs=[nc.scalar.lower_ap(out_ap)],
            ))

    with TileContext(nc) as tc:
        with contextlib.ExitStack() as ctx:
            singles = ctx.enter_context(tc.tile_pool(name="singles", bufs=1))
            s1 = ctx.enter_context(tc.tile_pool(name="s1", bufs=1))
            tfp = ctx.enter_context(tc.tile_pool(name="tfp", bufs=3))
            plp = ctx.enter_context(tc.tile_pool(name="plp", bufs=6))
            rtp = ctx.enter_context(tc.tile_pool(name="rtp", bufs=6))
            fin = ctx.enter_context(tc.tile_pool(name="fin", bufs=2))
            gpool = ctx.enter_context(
                tc.tile_pool(name="gpool", bufs=1, space="PSUM"))
            apool = ctx.enter_context(
                tc.tile_pool(name="apool", bufs=2, space="PSUM"))

            cc32 = singles.tile([128, 2, 384], f32, tag="cc32")
            nc.gpsimd.dma_start(out=cc32[:], in_=cc_dram.ap())
            ccb = singles.tile([128, 2, 384], bf16, tag="ccb")
            nc.vector.tensor_copy(out=ccb[:].rearrange("p a b -> p (a b)"),
                                  in_=cc32[:].rearrange("p a b -> p (a b)"))

            # ---------------- stage 1: per-pixel features (both images) -----
            feats = []
            xy = [x_dram, y_dram]
            for ui in range(2):
                X = s1.tile([128, 3, NCHUNK], f32, tag=f"X{ui}")
                src = xy[ui].ap().rearrange("c (p t) -> c p t", p=128)
                for ch in range(3):
                    nc.gpsimd.dma_start(out=X[:, ch, :], in_=src[ch])
                L = s1.tile([128, 3, NCHUNK], f32, tag=f"L{ui}")
                SQ = s1.tile([128, 3, NCHUNK], f32, tag=f"SQ{ui}")
                for ch in range(3):
                    nc.scalar.activation(out=L[:, ch, :], in_=X[:, ch, :],
                                         func=AF.Ln, bias=float(EPS))
                    nc.scalar.activation(out=SQ[:, ch, :], in_=X[:, ch, :],
                                         func=AF.Square, bias=float(EPS))
                SS = s1.tile([128, NCHUNK], f32, tag=f"SS{ui}")
                nc.gpsimd.tensor_add(SS[:], SQ[:, 0, :], SQ[:, 1, :])
                nc.gpsimd.tensor_add(SS[:], SS[:], SQ[:, 2, :])
                IY = s1.tile([128, NCHUNK], f32, tag=f"IY{ui}")
                nc.scalar.activation(out=IY[:], in_=SS[:], func=AF.Sqrt)

                U = s1.tile([128, NCHUNK], f32, tag=f"U{ui}")
                W = s1.tile([128, NCHUNK], f32, tag=f"W{ui}")
                V = s1.tile([128, NCHUNK], f32, tag=f"V{ui}")
                nc.vector.tensor_sub(U[:], L[:, 0, :], L[:, 1, :])
                nc.vector.tensor_sub(W[:], L[:, 1, :], L[:, 2, :])
                nc.vector.tensor_sub(V[:], L[:, 0, :], L[:, 2, :])

                FEAT = s1.tile([128, NBLK, 32, CB], bf16, tag=f"FEAT{ui}")
                for s in _S_ONES:
                    nc.gpsimd.memset(FEAT[:, :, s, :], 1.0)
                for s in _S_ZERO:
                    nc.gpsimd.memset(FEAT[:, :, s, :], 0.0)

                def slot(sl):
                    return FEAT[:, :, sl, :]

                def r512(t):
                    return t[:].rearrange("p (a b) -> p a b", a=NBLK)

                SD = s1.tile([128, NCHUNK], f32, tag=f"SD{ui}")
                QA = s1.tile([128, NCHUNK], f32, tag=f"QA{ui}")
                BD = s1.tile([128, NCHUNK], f32, tag=f"BD{ui}")
                TMP = s1.tile([128, NCHUNK], f32, tag=f"TMP{ui}")
                TMP2 = s1.tile([128, NCHUNK], f32, tag=f"TMP2{ui}")
                for f, dd in ((0, U), (1, W), (2, V)):
                    b = _F_SLOTS[f]
                    # residual chains run on the (otherwise idle) Pool engine
                    eng = nc.gpsimd
                    # SD = 50*d ; QA = 2500*d^2 ; BD = 100*d
                    nc.vector.tensor_scalar_mul(out=SD[:], in0=dd[:],
                                                scalar1=K_SC)
                    nc.scalar.activation(out=QA[:], in_=SD[:],
                                         func=AF.Square)
                    nc.vector.tensor_scalar_mul(out=BD[:], in0=SD[:],
                                                scalar1=2.0)
                    # hilo3 of QA -> slots A1..A3 (hi lands in the slot via a
                    # strided cast copy; lo terms via TT subtracts)
                    nc.vector.tensor_copy(out=slot(b + _S_A[0]),
                                          in_=r512(QA))
                    eng.tensor_tensor(
                        out=slot(b + _S_A[1]), in0=r512(QA),
                        in1=slot(b + _S_A[0]), op=ALU.subtract)
                    eng.tensor_tensor(out=r512(TMP), in0=slot(b + _S_A[0]),
                                      in1=slot(b + _S_A[1]), op=ALU.add)
                    eng.tensor_tensor(
                        out=slot(b + _S_A[2]), in0=r512(QA),
                        in1=r512(TMP), op=ALU.subtract)
                    # hilo3 of BD -> Bh (x3 slots), Bl (x2), Bl2
                    nc.vector.tensor_copy(out=slot(b + _S_BH[0]),
                                          in_=r512(BD))
                    for s in _S_BH[1:]:
                        nc.vector.tensor_copy(out=slot(b + s),
                                              in_=slot(b + _S_BH[0]))
                    eng.tensor_tensor(
                        out=slot(b + _S_BL[0]), in0=r512(BD),
                        in1=slot(b + _S_BH[0]), op=ALU.subtract)
                    nc.vector.tensor_copy(out=slot(b + _S_BL[1]),
                                          in_=slot(b + _S_BL[0]))
                    eng.tensor_tensor(out=r512(TMP2), in0=slot(b + _S_BH[0]),
                                      in1=slot(b + _S_BL[0]), op=ALU.add)
                    eng.tensor_tensor(
                        out=slot(b + _S_BL2[0]), in0=r512(BD),
                        in1=r512(TMP2), op=ALU.subtract)
                feats.append((FEAT, IY))

            # ---------------- stage 2: pair groups --------------------------
            NPAIR = NCHUNK // 2          # 256 pairs per image
            GRP = 3                      # pairs per elementwise group
            units = []
            for ui in range(2):
                FEAT, IY = feats[ui]
                G = gpool.tile([128, 128], f32, tag=f"G{ui}")
                units.append(G)
                pend_q = []
                TF = None
                tf_sb = -1

                def flush(pend):
                    RTWp, PLp, c0_, nchp = pend
                    for c in range(nchp):
                        ch_g = c0_ + c
                        nc.tensor.matmul(
                            out=G[:],
                            lhsT=RTWp[:, c, :],
                            rhs=PLp[:, c, :],
                            start=(ch_g == 0), stop=(ch_g == NCHUNK - 1),
                            skip_group_check=True)

                chunk = 0
                p = 0
                gidx = 0
                while p < NPAIR:
                    npair = min(GRP, NPAIR - p)
                    nch = 2 * npair
                    A = apool.tile([128, 6, 256], f32, tag="A")
                    PL = plp.tile([128, 6, 128], bf16, tag="PL")
                    RTW = rtp.tile([128, 6, 128], bf16, tag="RTW")
                    for jj in range(npair):
                        pg = p + jj
                        blk = pg // 2
                        sb = blk // NBT
                        if sb != tf_sb:
                            TF = tfp.tile([128, NBT, 128], bf16, tag="TF")
                            nc.sync.dma_start_transpose(
                                out=TF[:],
                                in_=FEAT[:, sb * NBT:(sb + 1) * NBT]
                                    .rearrange("p a s c -> p (a s c)"))
                            tf_sb = sb
                        nc.tensor.matmul(
                            out=A[:, 2 * jj:2 * jj + 2, 0:192],
                            lhsT=TF[:, blk % NBT, :],
                            rhs=ccb[:, pg % 2, :],
                            start=True, stop=True)
                    # run the PE two groups behind the matmuls feeding it
                    if len(pend_q) >= 2:
                        flush(pend_q.pop(0))

                    # weighted pair: RTW = [iy*Pu | iy*Pw], one DVE pass
                    nc.vector._custom_dve(
                        OP_M, out=RTW[:, 0:nch, :],
                        in0=A[:, 0:nch, 0:128],
                        in1=IY[:, chunk:chunk + nch].unsqueeze(2)
                            .broadcast_to([128, nch, 128]),
                        s0=CH0, s1=CH1)
                    # plain pair (w, v): 1-pass Reciprocal
                    if PLAIN_DVE_MOD and (gidx % PLAIN_DVE_MOD
                                          == PLAIN_DVE_MOD - 1):
                        nc.vector._custom_dve(
                            OP_PLAIN, out=PL[:, 0:nch, :],
                            in0=A[:, 0:nch, 64:192],
                            s0=CH0, s1=CH1)
                    else:
                        direct_recip(PL[:, 0:nch, :],
                                     A[:, 0:nch, 64:192], 0.0)

                    pend_q.append((RTW, PL, chunk, nch))
                    chunk += nch
                    p += npair
                    gidx += 1
                for pend in pend_q:
                    flush(pend)

            # ---------------- stage 3: normalize + Hellinger ----------------
            SQs = []
            for ui in range(2):
                G = units[ui]
                red = fin.tile([128, 1], f32, tag=f"red{ui}")
                nc.vector.tensor_reduce(out=red[0:64, :], in_=G[0:64, :],
                                        axis=AX.X, op=ALU.add)
                nc.vector.tensor_reduce(out=red[64:128, :],
                                        in_=G[64:128, 64:128],
                                        axis=AX.X, op=ALU.add)
                tot = fin.tile([1, 1], f32, tag=f"tot{ui}")
                nc.gpsimd.tensor_reduce(out=tot[:], in_=red[:], axis=AX.C,
                                        op=ALU.add)
                inv = fin.tile([1, 1], f32, tag=f"inv{ui}")
                nc.vector.reciprocal(out=inv[:], in_=tot[:])
                invb = fin.tile([128, 1], f32, tag=f"invb{ui}")
                nc.gpsimd.partition_broadcast(invb[:], inv[:])
                SQt = fin.tile([128, 128], f32, tag=f"SQt{ui}")
                nc.scalar.activation(out=SQt[0:64, :], in_=G[0:64, :],
                                     func=AF.Sqrt, scale=invb[0:64, 0:1])
                nc.scalar.activation(out=SQt[64:128, 64:128],
                                     in_=G[64:128, 64:128],
                                     func=AF.Sqrt, scale=invb[64:128, 0:1])
                SQs.append(SQt)

            DF = fin.tile([128, 128], f32, tag="DF")
            nc.vector.tensor_sub(DF[0:64, :], SQs[1][0:64, :],
                                 SQs[0][0:64, :])
            nc.vector.tensor_sub(DF[64:128, 64:128],
                                 SQs[1][64:128, 64:128],
                                 SQs[0][64:128, 64:128])
            SC2 = fin.tile([128, 128], f32, tag="SC2")
            acc = fin.tile([128, 1], f32, tag="acc")
            nc.scalar.activation(out=SC2[0:64, :], in_=DF[0:64, :],
                                 func=AF.Square, accum_out=acc[0:64, :])
            nc.scalar.activation(out=SC2[64:128, 64:128],
                                 in_=DF[64:128, 64:128],
                                 func=AF.Square, accum_out=acc[64:128, :])
            htot = fin.tile([1, 1], f32, tag="htot")
            nc.gpsimd.tensor_reduce(out=htot[:], in_=acc[:], axis=AX.C,
                                    op=ALU.add)
            hres = fin.tile([1, 1], f32, tag="hres")
            nc.scalar.activation(out=hres[:], in_=htot[:], func=AF.Sqrt,
                                 scale=0.5)
            nc.sync.dma_start(out=h_dram.ap(), in_=hres[:])

    nc.finalize()
    return nc


def _get_module():
    if "nc" not in _CACHE:
        _CACHE["nc"] = _build_module()
    return _CACHE["nc"]


def _run(x, y, trace=False):
    from concourse.bass_utils import run_bass_kernel_spmd
    nc = _get_module()
    x = np.ascontiguousarray(np.asarray(x, np.float32).reshape(8, 3, NPIX))
    y = np.ascontiguousarray(np.asarray(y, np.float32).reshape(8, 3, NPIX))
    in_maps = [{"x_img": x[i], "y_img": y[i]} for i in range(N_CORES)]
    res = run_bass_kernel_spmd(nc, in_maps, core_ids=list(range(N_CORES)),
                               trace=trace)
    hs = np.array([res.results[i]["h_out"].reshape(-1)[0]
                   for i in range(N_CORES)], np.float64)
    return hs, res


def kernel(x, y):
    hs, _ = _run(x, y)
    return np.float32(hs.mean())


# revision 25
# speedup vs baseline: 1.4934x; 1.0291x over previous
"""Trainium2 Bass kernel for nn_ColorHistogramMatchingLoss (v6).

Data-parallel over batch: core i processes image pair (x[i], y[i]) and emits
the per-image Hellinger distance; the host averages 8 scalars.

v6 reformulation (validated in numpy):
  - The three histograms reduce (via flip/transpose invariance of the loss)
    to G_uv = (iy Ru)^T Rv, G_uw = (iy Ru)^T Rw, G_wv = (iy Rw)^T Rv over the
    log-ratio fields u=lr-lg, w=lg-lb, v=lr-lb, and the weighted RBF column
    is iy * plain Lorentzian of the same field.
  - The PE matmul produces X = s^2 + 1 DIRECTLY (s = (d-c)/0.02) via a
    cancellation-safe split-quadratic row basis: per field 9 bf16 rows
    {hilo3(2500 d^2) x coeff 1, hilo3(100 d) x split coeffs of cp,
    cross-level products} plus 3 shared ones-rows carrying hilo3(cp^2+1),
    where cp = (31.5-k)/0.21.  Max relative error on r = 1/X is ~0.4%.
  - Elementwise then needs only a RECIPROCAL: DVE 1-pass custom Newton
    (LORENTZRM fuses the iy weight via broadcast Src1) and ScalarE 1-pass
    Reciprocal activation for the plain (w,v) pair.
  - G accumulates per chunk: G += RTW_c^T PL_c giving quadrants
    [G_uw | G_uv; junk | G_wv]; normalization cancels all global scales.
"""

import numpy as np

P = 128
NCHUNK = 512          # 128-pixel chunks per image
NPIX = 65536
D = 64
EPS = 1e-6
N_CORES = 8
CB = 4                # chunks per block
NBLK = NCHUNK // CB   # 128
NBT = 8               # blocks per batched transpose
CH0, CH1 = -0.23549792, 2.0017324   # Chebyshev recip seed constants
K_SC = 50.0           # s = 50*d + cp

# plain (w,v) Lorentzian pair routing: every PLAIN_DVE_MOD-th group on DVE
# (1-pass custom), rest on ScalarE 1-pass Reciprocal. 0 disables DVE routing.
PLAIN_DVE_MOD = 0

# slot layout (32 slots x CB=4 chunks; TF row = 4*s + c)
_F_SLOTS = {0: 0, 1: 9, 2: 18}       # base slot per field
_S_A = (0, 1, 2)                     # hilo3(2500 d^2), coeff 1
_S_BH = (3, 4, 5)                    # bf16(100 d) x {cph, cpl, cpl2}
_S_BL = (6, 7)                       # lo(100 d)   x {cph, cpl}
_S_BL2 = (8,)                        # lo2(100 d)  x {cph}
_S_ONES = (27, 28, 29)               # 1.0 x hilo3(cp^2 + 1)
_S_ZERO = (30, 31)

_CACHE = {}


def _bf16(a):
    a = np.asarray(a, np.float32)
    x32 = a.view(np.uint32)
    r = ((x32 + 0x7fff + ((x32 >> 16) & 1)) & 0xFFFF0000).astype(np.uint32)
    return r.view(np.float32)


def _register_dve_ops():
    import concourse.dve_ops as dve_ops
    if "LORENTZR" in dve_ops._SUB_OPCODE_FOR_NAME:
        ops = {o.name: o for o in dve_ops.OPS}
        return ops["LORENTZR"], ops["LORENTZRM"]
    from concourse.dve_spec import Spec, Src0, Src1, C0, C1, AluOp, Bin
    from concourse.dve_spec import lower, _has_src1
    from concourse.dve_uop import DveOpSpec

    def _mk(name, body, ref):
        spec = Spec(body=body, reference=ref)
        row = dve_ops._CUSTOM_DVE_ROW_BASE + len(dve_ops.OPS)
        shas = {}
        for ver in ("v3", "v4"):
            tmp = DveOpSpec(name=name, opcode=row,
                            uops=lower(spec, ver=ver), rd1_en=_has_src1(spec))
            shas[ver] = tmp.sha(ver)
        op = dve_ops.DveOp(name, spec, subdim=False, uops_sha=shas)
        dve_ops.OPS.append(op)
        dve_ops.CUSTOM_DVE_SPECS[name] = spec
        dve_ops._SUB_OPCODE_FOR_NAME[name] = row
        return op

    def _recip1nr(xx):
        nxx = (~xx.view(np.int32)).view(np.float32)
        y0 = nxx * np.float32(CH0)
        return y0 * (np.float32(CH1) - xx * y0)

    # plain: out = recip(in0)   (in0 = s^2+1 from the matmul)
    n1 = Bin(AluOp.BITWISE_NOT, Src0, Src0)
    y1 = n1 * C0
    op_plain = _mk(
        "LORENTZR", y1 * (C1 - Src0 * y1),
        lambda in0, in1, s0, s1, imm2: _recip1nr(in0.astype(np.float32)))

    # weighted: out = Src1 * recip(in0)   (Src1 = broadcast iy column)
    n2 = Bin(AluOp.BITWISE_NOT, Src0, Src0)
    y2 = n2 * C0
    op_m = _mk(
        "LORENTZRM", (y2 * (C1 - Src0 * y2)) * Src1,
        lambda in0, in1, s0, s1, imm2:
            in1.astype(np.float32) * _recip1nr(in0.astype(np.float32)))
    return op_plain, op_m


def _build_cc():
    """cc coefficient tensor [128, 2, 384] fp32 (cast to bf16 on chip).

    Row = 4*slot + chunk-in-block. Pair m covers chunks (2m, 2m+1); its 384
    columns are col = j*192 + f*64 + k, producing
    X = s^2 + 1 = 2500 d_f^2 + cp_k*(100 d_f) + (cp_k^2 + 1).
    """
    cp = (31.5 - np.arange(D, dtype=np.float64)) / 0.21
    cph = _bf16(cp)
    cpl = _bf16(cp - np.float64(cph))
    cpl2 = _bf16(cp - np.float64(cph) - np.float64(cpl))
    c0 = cp * cp + 1.0
    c0h = _bf16(c0)
    c0l = _bf16(c0 - np.float64(c0h))
    c0l2 = _bf16(c0 - np.float64(c0h) - np.float64(c0l))

    cc = np.zeros((128, 2, 384), np.float32)
    for m in range(2):
        for j in range(2):
            c = 2 * m + j
            for f in range(3):
                o = j * 192 + f * 64
                b = _F_SLOTS[f]
                for s in _S_A:
                    cc[4 * (b + s) + c, m, o:o + 64] = 1.0
                for s, cf in zip(_S_BH, (cph, cpl, cpl2)):
                    cc[4 * (b + s) + c, m, o:o + 64] = cf
                for s, cf in zip(_S_BL, (cph, cpl)):
                    cc[4 * (b + s) + c, m, o:o + 64] = cf
                cc[4 * (b + _S_BL2[0]) + c, m, o:o + 64] = cph
            # shared ones rows span all 3 fields' columns
            for s, cf in zip(_S_ONES, (c0h, c0l, c0l2)):
                for f in range(3):
                    o = j * 192 + f * 64
                    cc[4 * s + c, m, o:o + 64] = cf
    return cc


def _build_module():
    import concourse.bass as bass
    import concourse.mybir as mybir
    from concourse import bacc
    from concourse.tile import TileContext
    import contextlib

    OP_PLAIN, OP_M = _register_dve_ops()

    f32 = mybir.dt.float32
    bf16 = mybir.dt.bfloat16
    AF = mybir.ActivationFunctionType
    ALU = mybir.AluOpType
    AX = mybir.AxisListType

    nc = bacc.Bacc("TRN2", target_bir_lowering=False, debug=False,
                   num_devices=N_CORES)

    x_dram = nc.dram_tensor("x_img", (3, NPIX), f32, kind="ExternalInput")
    y_dram = nc.dram_tensor("y_img", (3, NPIX), f32, kind="ExternalInput")
    h_dram = nc.dram_tensor("h_out", (1, 1), f32, kind="ExternalOutput")
    cc_np = _build_cc()
    cc_dram = nc.inline_tensor(cc_np, name="cc_const")

    for val in (float(EPS), 0.0):
        t = nc.alloc_sbuf_tensor(f"const-{val}", [128, 1], f32)
        nc.gpsimd.memset(t.ap(), val)
        nc.const_aps.aps[(f32, float(val))] = t.ap()
    nc.all_engine_barrier()

    def direct_recip(out_ap, in_ap, bias):
        imm = lambda v: mybir.ImmediateValue(dtype=f32, value=float(v))
        nc.scalar.add_instruction(
            mybir.InstActivation(
                name=nc.get_next_instruction_name(),
                func=AF.Reciprocal,
                ins=[nc.scalar.lower_ap(in_ap), imm(bias), imm(1.0),
                     imm(0.0)],
                outs=[nc.scalar.lower_ap(out_ap)],
            ))

    with TileContext(nc) as tc:
        with contextlib.ExitStack() as ctx:
            singles = ctx.enter_context(tc.tile_pool(name="singles", bufs=1))
            s1 = ctx.enter_context(tc.tile_pool(name="s1", bufs=1))
            tfp = ctx.enter_context(tc.tile_pool(name="tfp", bufs=3))
            plp = ctx.enter_context(tc.tile_pool(name="plp", bufs=6))
            rtp = ctx.enter_context(tc.tile_pool(name="rtp", bufs=6))
            fin = ctx.enter_context(tc.tile_pool(name="fin", bufs=2))
            gpool = ctx.enter_context(
                tc.tile_pool(name="gpool", bufs=1, space="PSUM"))
            apool = ctx.enter_context(
                tc.tile_pool(name="apool", bufs=2, space="PSUM"))

            cc32 = singles.tile([128, 2, 384], f32, tag="cc32")
            nc.gpsimd.dma_start(out=cc32[:], in_=cc_dram.ap())
            ccb = singles.tile([128, 2, 384], bf16, tag="ccb")
            nc.vector.tensor_copy(out=ccb[:].rearrange("p a b -> p (a b)"),
                                  in_=cc32[:].rearrange("p a b -> p (a b)"))

            # ---------------- stage 1: per-pixel features (both images) -----
            # Phase A: everything the activation tables touch, batched so the
            # ACT table loads once per function set; produces QA_f = 2500 d^2
            # and BD_f = 100 d per field.
            feats = []
            prep = []
            xy = [x_dram, y_dram]
            for ui in range(2):
                X = s1.tile([128, 3, NCHUNK], f32, tag=f"X{ui}")
                src = xy[ui].ap().rearrange("c (p t) -> c p t", p=128)
                for ch in range(3):
                    nc.gpsimd.dma_start(out=X[:, ch, :], in_=src[ch])
                L = s1.tile([128, 3, NCHUNK], f32, tag=f"L{ui}")
                SQ = s1.tile([128, 3, NCHUNK], f32, tag=f"SQ{ui}")
                for ch in range(3):
                    nc.scalar.activation(out=L[:, ch, :], in_=X[:, ch, :],
                                         func=AF.Ln, bias=float(EPS))
                    nc.scalar.activation(out=SQ[:, ch, :], in_=X[:, ch, :],
                                         func=AF.Square, bias=float(EPS))
                SS = s1.tile([128, NCHUNK], f32, tag=f"SS{ui}")
                nc.gpsimd.tensor_add(SS[:], SQ[:, 0, :], SQ[:, 1, :])
                nc.gpsimd.tensor_add(SS[:], SS[:], SQ[:, 2, :])
                IY = s1.tile([128, NCHUNK], f32, tag=f"IY{ui}")
                nc.scalar.activation(out=IY[:], in_=SS[:], func=AF.Sqrt)

                U = s1.tile([128, NCHUNK], f32, tag=f"U{ui}")
                W = s1.tile([128, NCHUNK], f32, tag=f"W{ui}")
                V = s1.tile([128, NCHUNK], f32, tag=f"V{ui}")
                nc.vector.tensor_sub(U[:], L[:, 0, :], L[:, 1, :])
                nc.vector.tensor_sub(W[:], L[:, 1, :], L[:, 2, :])
                nc.vector.tensor_sub(V[:], L[:, 0, :], L[:, 2, :])

                FEAT = s1.tile([128, NBLK, 32, CB], bf16, tag=f"FEAT{ui}")
                for s in _S_ONES:
                    nc.gpsimd.memset(FEAT[:, :, s, :], 1.0)
                for s in _S_ZERO:
                    nc.gpsimd.memset(FEAT[:, :, s, :], 0.0)

                fqb = []
                for f, dd in ((0, U), (1, W), (2, V)):
                    SD = s1.tile([128, NCHUNK], f32, tag=f"SD{ui}")
                    QA = s1.tile([128, NCHUNK], f32, tag=f"QA{ui}{f}")
                    BD = s1.tile([128, NCHUNK], f32, tag=f"BD{ui}{f}")
                    nc.vector.tensor_scalar_mul(out=SD[:], in0=dd[:],
                                                scalar1=K_SC)
                    nc.scalar.activation(out=QA[:], in_=SD[:],
                                         func=AF.Square)
                    nc.vector.tensor_scalar_mul(out=BD[:], in0=SD[:],
                                                scalar1=2.0)
                    fqb.append((QA, BD))
                feats.append((FEAT, IY))
                prep.append(fqb)

            # Phase B: hilo3 residual chains into the FEAT slots. Image 0 is
            # latency-critical (gates stage 2), so its chains spread across
            # DVE and Pool; image 1 runs entirely on Pool, overlapped with
            # image 0's stage 2.
            for ui in range(2):
                FEAT, IY = feats[ui]

                def slot(sl):
                    return FEAT[:, :, sl, :]

                def r512(t):
                    return t[:].rearrange("p (a b) -> p a b", a=NBLK)

                for f in range(3):
                    QA, BD = prep[ui][f]
                    b = _F_SLOTS[f]
                    if ui == 0:
                        eng = nc.vector if f == 1 else nc.gpsimd
                        cp_eng = nc.vector
                    else:
                        eng = nc.gpsimd
                        cp_eng = nc.gpsimd
                    TMP = s1.tile([128, NCHUNK], f32, tag=f"TMP{ui}")
                    TMP2 = s1.tile([128, NCHUNK], f32, tag=f"TMP2{ui}")
                    # hilo3 of QA -> slots A1..A3
                    cp_eng.tensor_copy(out=slot(b + _S_A[0]), in_=r512(QA))
                    eng.tensor_tensor(
                        out=slot(b + _S_A[1]), in0=r512(QA),
                        in1=slot(b + _S_A[0]), op=ALU.subtract)
                    eng.tensor_tensor(out=r512(TMP), in0=slot(b + _S_A[0]),
                                      in1=slot(b + _S_A[1]), op=ALU.add)
                    eng.tensor_tensor(
                        out=slot(b + _S_A[2]), in0=r512(QA),
                        in1=r512(TMP), op=ALU.subtract)
                    # hilo3 of BD -> Bh (x3 slots), Bl (x2), Bl2
                    cp_eng.tensor_copy(out=slot(b + _S_BH[0]), in_=r512(BD))
                    for s in _S_BH[1:]:
                        cp_eng.tensor_copy(out=slot(b + s),
                                           in_=slot(b + _S_BH[0]))
                    eng.tensor_tensor(
                        out=slot(b + _S_BL[0]), in0=r512(BD),
                        in1=slot(b + _S_BH[0]), op=ALU.subtract)
                    cp_eng.tensor_copy(out=slot(b + _S_BL[1]),
                                       in_=slot(b + _S_BL[0]))
                    eng.tensor_tensor(out=r512(TMP2), in0=slot(b + _S_BH[0]),
                                      in1=slot(b + _S_BL[0]), op=ALU.add)
                    eng.tensor_tensor(
                        out=slot(b + _S_BL2[0]), in0=r512(BD),
                        in1=r512(TMP2), op=ALU.subtract)

            # ---------------- stage 2: pair groups --------------------------
            NPAIR = NCHUNK // 2          # 256 pairs per image
            GRP = 3                      # pairs per elementwise group
            units = []
            for ui in range(2):
                FEAT, IY = feats[ui]
                G = gpool.tile([128, 128], f32, tag=f"G{ui}")
                units.append(G)
                pend_q = []
                TF = None
                tf_sb = -1

                def flush(pend):
                    RTWp, PLp, c0_, nchp = pend
                    for c in range(nchp):
                        ch_g = c0_ + c
                        nc.tensor.matmul(
                            out=G[:],
                            lhsT=RTWp[:, c, :],
                            rhs=PLp[:, c, :],
                            start=(ch_g == 0), stop=(ch_g == NCHUNK - 1),
                            skip_group_check=True)

                chunk = 0
                p = 0
                gidx = 0
                while p < NPAIR:
                    npair = min(GRP, NPAIR - p)
                    nch = 2 * npair
                    A = apool.tile([128, 6, 256], f32, tag="A")
                    PL = plp.tile([128, 6, 128], bf16, tag="PL")
                    RTW = rtp.tile([128, 6, 128], bf16, tag="RTW")
                    for jj in range(npair):
                        pg = p + jj
                        blk = pg // 2
                        sb = blk // NBT
                        if sb != tf_sb:
                            TF = tfp.tile([128, NBT, 128], bf16, tag="TF")
                            nc.sync.dma_start_transpose(
                                out=TF[:],
                                in_=FEAT[:, sb * NBT:(sb + 1) * NBT]
                                    .rearrange("p a s c -> p (a s c)"))
                            tf_sb = sb
                        nc.tensor.matmul(
                            out=A[:, 2 * jj:2 * jj + 2, 0:192],
                            lhsT=TF[:, blk % NBT, :],
                            rhs=ccb[:, pg % 2, :],
                            start=True, stop=True)
                    # run the PE two groups behind the matmuls feeding it
                    if len(pend_q) >= 2:
                        flush(pend_q.pop(0))

                    # weighted pair: RTW = [iy*Pu | iy*Pw], one DVE pass
                    nc.vector._custom_dve(
                        OP_M, out=RTW[:, 0:nch, :],
                        in0=A[:, 0:nch, 0:128],
                        in1=IY[:, chunk:chunk + nch].unsqueeze(2)
                            .broadcast_to([128, nch, 128]),
                        s0=CH0, s1=CH1)
                    # plain pair (w, v): 1-pass Reciprocal
                    if PLAIN_DVE_MOD and (gidx % PLAIN_DVE_MOD
                                          == PLAIN_DVE_MOD - 1):
                        nc.vector._custom_dve(
                            OP_PLAIN, out=PL[:, 0:nch, :],
                            in0=A[:, 0:nch, 64:192],
                            s0=CH0, s1=CH1)
                    else:
                        direct_recip(PL[:, 0:nch, :],
                                     A[:, 0:nch, 64:192], 0.0)

                    pend_q.append((RTW, PL, chunk, nch))
                    chunk += nch
                    p += npair
                    gidx += 1
                for pend in pend_q:
                    flush(pend)

            # ---------------- stage 3: normalize + Hellinger ----------------
            SQs = []
            for ui in range(2):
                G = units[ui]
                red = fin.tile([128, 1], f32, tag=f"red{ui}")
                nc.vector.tensor_reduce(out=red[0:64, :], in_=G[0:64, :],
                                        axis=AX.X, op=ALU.add)
                nc.vector.tensor_reduce(out=red[64:128, :],
                                        in_=G[64:128, 64:128],
                                        axis=AX.X, op=ALU.add)
                tot = fin.tile([1, 1], f32, tag=f"tot{ui}")
                nc.gpsimd.tensor_reduce(out=tot[:], in_=red[:], axis=AX.C,
                                        op=ALU.add)
                inv = fin.tile([1, 1], f32, tag=f"inv{ui}")
                nc.vector.reciprocal(out=inv[:], in_=tot[:])
                invb = fin.tile([128, 1], f32, tag=f"invb{ui}")
                nc.gpsimd.partition_broadcast(invb[:], inv[:])
                SQt = fin.tile([128, 128], f32, tag=f"SQt{ui}")
                nc.scalar.activation(out=SQt[0:64, :], in_=G[0:64, :],
                                     func=AF.Sqrt, scale=invb[0:64, 0:1])
                nc.scalar.activation(out=SQt[64:128, 64:128],
                                     in_=G[64:128, 64:128],
                                     func=AF.Sqrt, scale=invb[64:128, 0:1])
                SQs.append(SQt)

            DF = fin.tile([128, 128], f32, tag="DF")
            nc.vector.tensor_sub(DF[0:64, :], SQs[1][0:64, :],
                                 SQs[0][0:64, :])
            nc.vector.tensor_sub(DF[64:128, 64:128],
                                 SQs[1][64:128, 64:128],
                                 SQs[0][64:128, 64:128])
            SC2 = fin.tile([128, 128], f32, tag="SC2")
            acc = fin.tile([128, 1], f32, tag="acc")
            nc.scalar.activation(out=SC2[0:64, :], in_=DF[0:64, :],
                                 func=AF.Square, accum_out=acc[0:64, :])
            nc.scalar.activation(out=SC2[64:128, 64:128],
                                 in_=DF[64:128, 64:128],
                                 func=AF.Square, accum_out=acc[64:128, :])
            htot = fin.tile([1, 1], f32, tag="htot")
            nc.gpsimd.tensor_reduce(out=htot[:], in_=acc[:], axis=AX.C,
                                    op=ALU.add)
            hres = fin.tile([1, 1], f32, tag="hres")
            nc.scalar.activation(out=hres[:], in_=htot[:], func=AF.Sqrt,
                                 scale=0.5)
            nc.sync.dma_start(out=h_dram.ap(), in_=hres[:])

    nc.finalize()
    return nc


def _get_module():
    if "nc" not in _CACHE:
        _CACHE["nc"] = _build_module()
    return _CACHE["nc"]


def _run(x, y, trace=False):
    from concourse.bass_utils import run_bass_kernel_spmd
    nc = _get_module()
    x = np.ascontiguousarray(np.asarray(x, np.float32).reshape(8, 3, NPIX))
    y = np.ascontiguousarray(np.asarray(y, np.float32).reshape(8, 3, NPIX))
    in_maps = [{"x_img": x[i], "y_img": y[i]} for i in range(N_CORES)]
    res = run_bass_kernel_spmd(nc, in_maps, core_ids=list(range(N_CORES)),
                               trace=trace)
    hs = np.array([res.results[i]["h_out"].reshape(-1)[0]
                   for i in range(N_CORES)], np.float64)
    return hs, res


def kernel(x, y):
    hs, _ = _run(x, y)
    return np.float32(hs.mean())


# revision 26
# speedup vs baseline: 1.5377x; 1.0297x over previous
"""Trainium2 Bass kernel for nn_ColorHistogramMatchingLoss (v2).

Data-parallel over batch: core i processes image pair (x[i], y[i]) and emits
the per-image Hellinger distance; the host averages 8 scalars.

v2 reformulation (validated in numpy, rel err ~3e-5 vs the jax reference):
  - The three histograms reduce (via flip/transpose invariance of the loss)
    to G_uv = (iy Ru)^T Rv, G_uw = (iy Ru)^T Rw, G_wv = (iy Rw)^T Rv over the
    log-ratio fields u=lr-lg, w=lg-lb, v=lr-lb.
  - Key identity: the weighted RBF column iy*r equals iy times the PLAIN
    Lorentzian of the same field, so the PE matmul only produces the 3 plain
    t = 10.5*d + (31.5-k) tensors (192 cols/chunk, one FEAT tile, one
    transpose per block), every Lorentzian r = recip(t^2 + 0.0441) has a
    SCALAR bias (routable to either DVE custom 1-pass or ScalarE
    Square+Reciprocal 2-pass), and the iy weighting becomes a per-chunk
    tensor_scalar multiply (DVE 4x perf mode / GpSimd).
  - hi/lo bf16 split of 10.5*d keeps fp32-grade t; centers are exact since
    10.5*c_k = k - 31.5 is exactly representable.
  - G accumulates per chunk: G += RTW_c^T PL_c[64:192] giving quadrants
    [G_uw | G_uv; junk | G_wv]; normalization cancels the global 22.676^2.
"""

import numpy as np

P = 128
NCHUNK = 512          # 128-pixel chunks per image
NPIX = 65536
D = 64
EPS = 1e-6
N_CORES = 8
CB = 16               # chunks per block (one transposed weight tile)
NBLK = NCHUNK // CB   # 32
CH0, CH1 = -0.23549792, 2.0017324   # Chebyshev recip seed constants
K_SC = 50.0           # 10.5/0.21: matmul emits s = (d - c)/0.02 directly

# Routing knob (tuned against the HW trace): groups with
# (gidx % PLAIN_DVE_MOD) == PLAIN_DVE_MOD - 1 run the plain (w,v) Lorentzian
# pair on DVE (1-pass custom); the rest on ScalarE (Square+Reciprocal
# 2-pass). Interleaved so consecutive groups alternate the slow engine.
PLAIN_DVE_MOD = 4

_CACHE = {}


def _register_dve_ops():
    import concourse.dve_ops as dve_ops
    if "LORENTZ1" in dve_ops._SUB_OPCODE_FOR_NAME:
        ops = {o.name: o for o in dve_ops.OPS}
        return ops["LORENTZ1"], ops["LORENTZM1"]
    from concourse.dve_spec import Spec, Src0, Src1, C0, C1, AluOp, Bin, sq
    from concourse.dve_spec import lower, _has_src1, One
    from concourse.dve_uop import DveOpSpec

    def _mk(name, body, ref):
        spec = Spec(body=body, reference=ref)
        row = dve_ops._CUSTOM_DVE_ROW_BASE + len(dve_ops.OPS)
        shas = {}
        for ver in ("v3", "v4"):
            tmp = DveOpSpec(name=name, opcode=row,
                            uops=lower(spec, ver=ver), rd1_en=_has_src1(spec))
            shas[ver] = tmp.sha(ver)
        op = dve_ops.DveOp(name, spec, subdim=False, uops_sha=shas)
        dve_ops.OPS.append(op)
        dve_ops.CUSTOM_DVE_SPECS[name] = spec
        dve_ops._SUB_OPCODE_FOR_NAME[name] = row
        return op

    def _recip1nr(xx):
        nxx = (~xx.view(np.int32)).view(np.float32)
        y0 = nxx * np.float32(CH0)
        return y0 * (np.float32(CH1) - xx * y0)

    # plain: out = recip(s^2 + 1)
    x1 = Bin(AluOp.ADD, sq(Src0), One)
    n1 = Bin(AluOp.BITWISE_NOT, x1, x1)
    y1 = n1 * C0
    op_plain = _mk(
        "LORENTZ1", y1 * (C1 - x1 * y1),
        lambda in0, in1, s0, s1, imm2:
            _recip1nr(in0.astype(np.float32) ** 2 + np.float32(1.0)))

    # weighted: out = Src1 * recip(s^2 + 1)  (Src1 = broadcast iy column)
    x2 = Bin(AluOp.ADD, sq(Src0), One)
    n2 = Bin(AluOp.BITWISE_NOT, x2, x2)
    y2 = n2 * C0
    op_m = _mk(
        "LORENTZM1", (y2 * (C1 - x2 * y2)) * Src1,
        lambda in0, in1, s0, s1, imm2:
            in1.astype(np.float32)
            * _recip1nr(in0.astype(np.float32) ** 2 + np.float32(1.0)))
    return op_plain, op_m


def _build_cc():
    """cc coefficient tensor [128, 8, 384] fp32 (cast to bf16 on chip).

    TF row order: row(s, c) = 16*s + c for slot s, chunk-in-block c.
    Slots: 0 u_hi, 1 u_lo, 2 w_hi, 3 w_lo, 4 v_hi, 5 v_lo, 6/7 ones rows
    carrying the hi/lo split of the center offsets.
    Pair m covers chunks (2m, 2m+1); its 384 columns are
    col = j*192 + f*64 + k (j chunk-in-pair, f field, k center), producing
    s = 50*d_f + (31.5 - k)/0.21 = (d_f - c_k)/0.02.
    """
    cpr = ((31.5 - np.arange(D, dtype=np.float64)) / 0.21).astype(np.float32)
    cph = cpr.astype(np.dtype(">f4"))  # placeholder; real bf16 split below
    def bf16(a):
        a = np.asarray(a, np.float32)
        x32 = a.view(np.uint32)
        r = ((x32 + 0x7fff + ((x32 >> 16) & 1)) & 0xFFFF0000).astype(np.uint32)
        return r.view(np.float32)
    cph = bf16(cpr)
    cpl = bf16(cpr - cph)
    cc = np.zeros((128, 8, 384), np.float32)
    for m in range(8):
        for j in range(2):
            c = 2 * m + j
            for f in range(3):
                o = j * 192 + f * 64
                cc[16 * (2 * f) + c, m, o:o + 64] = 1.0
                cc[16 * (2 * f + 1) + c, m, o:o + 64] = 1.0
                cc[16 * 6 + c, m, o:o + 64] = cph
                cc[16 * 7 + c, m, o:o + 64] = cpl
    return cc


def _build_module():
    import concourse.bass as bass
    import concourse.mybir as mybir
    from concourse import bacc
    from concourse.tile import TileContext
    import contextlib

    OP_PLAIN, OP_M = _register_dve_ops()

    f32 = mybir.dt.float32
    bf16 = mybir.dt.bfloat16
    AF = mybir.ActivationFunctionType
    ALU = mybir.AluOpType
    AX = mybir.AxisListType

    nc = bacc.Bacc("TRN2", target_bir_lowering=False, debug=False,
                   num_devices=N_CORES)

    x_dram = nc.dram_tensor("x_img", (3, NPIX), f32, kind="ExternalInput")
    y_dram = nc.dram_tensor("y_img", (3, NPIX), f32, kind="ExternalInput")
    h_dram = nc.dram_tensor("h_out", (1, 1), f32, kind="ExternalOutput")
    cc_np = _build_cc()
    cc_dram = nc.inline_tensor(cc_np, name="cc_const")

    # Pre-register scalar consts used as ACT bias so activations carry no
    # extra sem wait.
    for val in (float(EPS), 0.0):
        t = nc.alloc_sbuf_tensor(f"const-{val}", [128, 1], f32)
        nc.gpsimd.memset(t.ap(), val)
        nc.const_aps.aps[(f32, float(val))] = t.ap()
    nc.all_engine_barrier()

    def direct_recip(out_ap, in_ap, bias):
        # ScalarE Reciprocal activation (~0.4% max err on HW, fine at the
        # loss tolerance).
        imm = lambda v: mybir.ImmediateValue(dtype=f32, value=float(v))
        nc.scalar.add_instruction(
            mybir.InstActivation(
                name=nc.get_next_instruction_name(),
                func=AF.Reciprocal,
                ins=[nc.scalar.lower_ap(in_ap), imm(bias), imm(1.0),
                     imm(0.0)],
                outs=[nc.scalar.lower_ap(out_ap)],
            ))

    with TileContext(nc) as tc:
        with contextlib.ExitStack() as ctx:
            singles = ctx.enter_context(tc.tile_pool(name="singles", bufs=1))
            s1 = ctx.enter_context(tc.tile_pool(name="s1", bufs=1))
            tfp = ctx.enter_context(tc.tile_pool(name="tfp", bufs=4))
            plp = ctx.enter_context(tc.tile_pool(name="plp", bufs=6))
            rtp = ctx.enter_context(tc.tile_pool(name="rtp", bufs=6))
            qtp = ctx.enter_context(tc.tile_pool(name="qtp", bufs=4))
            fin = ctx.enter_context(tc.tile_pool(name="fin", bufs=2))
            gpool = ctx.enter_context(
                tc.tile_pool(name="gpool", bufs=1, space="PSUM"))
            apool = ctx.enter_context(
                tc.tile_pool(name="apool", bufs=3, space="PSUM"))

            cc32 = singles.tile([128, 8, 384], f32, tag="cc32")
            nc.gpsimd.dma_start(out=cc32[:], in_=cc_dram.ap())
            ccb = singles.tile([128, 8, 384], bf16, tag="ccb")
            nc.vector.tensor_copy(out=ccb[:].rearrange("p a b -> p (a b)"),
                                  in_=cc32[:].rearrange("p a b -> p (a b)"))

            # ---------------- stage 1: per-pixel features (both images) -----
            feats = []
            xy = [x_dram, y_dram]
            for ui in range(2):
                X = s1.tile([128, 3, NCHUNK], f32, tag=f"X{ui}")
                src = xy[ui].ap().rearrange("c (p t) -> c p t", p=128)
                for ch in range(3):
                    nc.gpsimd.dma_start(out=X[:, ch, :], in_=src[ch])
                L = s1.tile([128, 3, NCHUNK], f32, tag=f"L{ui}")
                SQ = s1.tile([128, 3, NCHUNK], f32, tag=f"SQ{ui}")
                for ch in range(3):
                    nc.scalar.activation(out=L[:, ch, :], in_=X[:, ch, :],
                                         func=AF.Ln, bias=float(EPS))
                    nc.scalar.activation(out=SQ[:, ch, :], in_=X[:, ch, :],
                                         func=AF.Square, bias=float(EPS))
                SS = s1.tile([128, NCHUNK], f32, tag=f"SS{ui}")
                nc.gpsimd.tensor_add(SS[:], SQ[:, 0, :], SQ[:, 1, :])
                nc.gpsimd.tensor_add(SS[:], SS[:], SQ[:, 2, :])
                IY = s1.tile([128, NCHUNK], f32, tag=f"IY{ui}")
                nc.scalar.activation(out=IY[:], in_=SS[:], func=AF.Sqrt)

                U = s1.tile([128, NCHUNK], f32, tag=f"U{ui}")
                W = s1.tile([128, NCHUNK], f32, tag=f"W{ui}")
                V = s1.tile([128, NCHUNK], f32, tag=f"V{ui}")
                nc.vector.tensor_sub(U[:], L[:, 0, :], L[:, 1, :])
                nc.vector.tensor_sub(W[:], L[:, 1, :], L[:, 2, :])
                nc.vector.tensor_sub(V[:], L[:, 0, :], L[:, 2, :])

                FEAT = s1.tile([128, NBLK, 8, CB], bf16, tag=f"FEAT{ui}")
                nc.gpsimd.memset(FEAT[:, :, 6, :], 1.0)
                nc.gpsimd.memset(FEAT[:, :, 7, :], 1.0)
                PH = s1.tile([128, NCHUNK], bf16, tag=f"PH{ui}")
                PH32 = s1.tile([128, NCHUNK], f32, tag=f"PH32{ui}")
                for f, dd in ((0, U), (1, W), (2, V)):
                    # hi = bf16(50*d); lo = bf16(50*d - hi)
                    nc.vector.tensor_scalar_mul(out=PH[:], in0=dd[:],
                                                scalar1=K_SC)
                    nc.sync.dma_start(
                        out=FEAT[:, :, 2 * f, :],
                        in_=PH[:].rearrange("p (a b) -> p a b", a=NBLK))
                    nc.vector.tensor_copy(out=PH32[:], in_=PH[:])
                    nc.vector.scalar_tensor_tensor(
                        out=FEAT[:, :, 2 * f + 1, :],
                        in0=dd[:].rearrange("p (a b) -> p a b", a=NBLK),
                        scalar=K_SC,
                        in1=PH32[:].rearrange("p (a b) -> p a b", a=NBLK),
                        op0=ALU.mult, op1=ALU.subtract)
                feats.append((FEAT, IY))

            # ---------------- stage 2: blocks -------------------------------
            units = []
            for ui in range(2):
                FEAT, IY = feats[ui]
                G = gpool.tile([128, 128], f32, tag=f"G{ui}")
                units.append(G)
                pending = None  # (RTW, PL, chunk0, nch) awaiting G matmuls
                gidx = 0
                chunk = 0

                def flush(pend):
                    if pend is None:
                        return
                    RTWp, PLp, c0, nchp = pend
                    for c in range(nchp):
                        ch_g = c0 + c
                        nc.tensor.matmul(
                            out=G[:],
                            lhsT=RTWp[:, c, :],
                            rhs=PLp[:, c, :],
                            start=(ch_g == 0), stop=(ch_g == NCHUNK - 1),
                            skip_group_check=True)

                for b in range(NBLK):
                    TF = tfp.tile([128, 128], bf16, tag="TF")
                    nc.sync.dma_start_transpose(
                        out=TF[:],
                        in_=FEAT[:, b].rearrange("p a b -> p (a b)"))
                    for m0 in (0, 2, 4, 6):
                        npair = 2
                        nch = 2 * npair
                        A = apool.tile([128, 4, 256], f32, tag="A")
                        PL = plp.tile([128, 4, 128], bf16, tag="PL")
                        RTW = rtp.tile([128, 4, 128], bf16, tag="RTW")
                        for j in range(npair):
                            nc.tensor.matmul(
                                out=A[:, 2 * j:2 * j + 2, 0:192],
                                lhsT=TF[:], rhs=ccb[:, m0 + j, :],
                                start=True, stop=True)
                        # previous group's G matmuls go after this group's
                        # mm1 so the PE always has ready work queued
                        flush(pending)

                        # weighted pair: RTW = [iy*Pu | iy*Pw] in one fused
                        # DVE pass (Src1 = broadcast iy column per chunk)
                        nc.vector._custom_dve(
                            OP_M, out=RTW[:, 0:nch, :],
                            in0=A[:, 0:nch, 0:128],
                            in1=IY[:, chunk:chunk + nch].unsqueeze(2)
                                .broadcast_to([128, nch, 128]),
                            s0=CH0, s1=CH1)
                        # plain pair (w, v) -> PL, routed DVE or ScalarE
                        if (gidx % PLAIN_DVE_MOD) == PLAIN_DVE_MOD - 1:
                            nc.vector._custom_dve(
                                OP_PLAIN, out=PL[:, 0:nch, :],
                                in0=A[:, 0:nch, 64:192],
                                s0=CH0, s1=CH1)
                        else:
                            QT = qtp.tile([128, 4, 128], bf16, tag="QT")
                            nc.scalar.activation(
                                out=QT[:, 0:nch, :],
                                in_=A[:, 0:nch, 64:192], func=AF.Square)
                            direct_recip(PL[:, 0:nch, :],
                                         QT[:, 0:nch, :], 1.0)

                        pending = (RTW, PL, chunk, nch)
                        gidx += 1
                        chunk += nch
                flush(pending)

            # ---------------- stage 3: normalize + Hellinger ----------------
            SQs = []
            for ui in range(2):
                G = units[ui]
                red = fin.tile([128, 1], f32, tag=f"red{ui}")
                nc.vector.tensor_reduce(out=red[0:64, :], in_=G[0:64, :],
                                        axis=AX.X, op=ALU.add)
                nc.vector.tensor_reduce(out=red[64:128, :],
                                        in_=G[64:128, 64:128],
                                        axis=AX.X, op=ALU.add)
                tot = fin.tile([1, 1], f32, tag=f"tot{ui}")
                nc.gpsimd.tensor_reduce(out=tot[:], in_=red[:], axis=AX.C,
                                        op=ALU.add)
                inv = fin.tile([1, 1], f32, tag=f"inv{ui}")
                nc.vector.reciprocal(out=inv[:], in_=tot[:])
                invb = fin.tile([128, 1], f32, tag=f"invb{ui}")
                nc.gpsimd.partition_broadcast(invb[:], inv[:])
                SQt = fin.tile([128, 128], f32, tag=f"SQt{ui}")
                nc.scalar.activation(out=SQt[0:64, :], in_=G[0:64, :],
                                     func=AF.Sqrt, scale=invb[0:64, 0:1])
                nc.scalar.activation(out=SQt[64:128, 64:128],
                                     in_=G[64:128, 64:128],
                                     func=AF.Sqrt, scale=invb[64:128, 0:1])
                SQs.append(SQt)

            DF = fin.tile([128, 128], f32, tag="DF")
            nc.vector.tensor_sub(DF[0:64, :], SQs[1][0:64, :],
                                 SQs[0][0:64, :])
            nc.vector.tensor_sub(DF[64:128, 64:128],
                                 SQs[1][64:128, 64:128],
                                 SQs[0][64:128, 64:128])
            SC2 = fin.tile([128, 128], f32, tag="SC2")
            acc = fin.tile([128, 1], f32, tag="acc")
            nc.scalar.activation(out=SC2[0:64, :], in_=DF[0:64, :],
                                 func=AF.Square, accum_out=acc[0:64, :])
            nc.scalar.activation(out=SC2[64:128, 64:128],
                                 in_=DF[64:128, 64:128],
                                 func=AF.Square, accum_out=acc[64:128, :])
            htot = fin.tile([1, 1], f32, tag="htot")
            nc.gpsimd.tensor_reduce(out=htot[:], in_=acc[:], axis=AX.C,
                                    op=ALU.add)
            hres = fin.tile([1, 1], f32, tag="hres")
            nc.scalar.activation(out=hres[:], in_=htot[:], func=AF.Sqrt,
                                 scale=0.5)
            nc.sync.dma_start(out=h_dram.ap(), in_=hres[:])

    nc.finalize()
    return nc


def _get_module():
    if "nc" not in _CACHE:
        _CACHE["nc"] = _build_module()
    return _CACHE["nc"]


def _run(x, y, trace=False):
    from concourse.bass_utils import run_bass_kernel_spmd
    nc = _get_module()
    x = np.ascontiguousarray(np.asarray(x, np.float32).reshape(8, 3, NPIX))
    y = np.ascontiguousarray(np.asarray(y, np.float32).reshape(8, 3, NPIX))
    in_maps = [{"x_img": x[i], "y_img": y[i]} for i in range(N_CORES)]
    res = run_bass_kernel_spmd(nc, in_maps, core_ids=list(range(N_CORES)),
                               trace=trace)
    hs = np.array([res.results[i]["h_out"].reshape(-1)[0]
                   for i in range(N_CORES)], np.float64)
    return hs, res


def kernel(x, y):
    hs, _ = _run(x, y)
    return np.float32(hs.mean())
